# revision 29
# baseline (speedup 1.0000x reference)
"""Trainium2 Bass kernel for 2-layer GAT (nn_GAT_47957604827269).

Strategy: partition nodes across 8 cores by dst range. Per layer:
  - per-core table slice build (local x^T tiles -> PE matmuls), AllGather to
    a full per-node feature table in DRAM: row = [as | (1,1,h)*H] in bf16,
    256B-aligned rows for the dma_gather ucode op.
  - per-edge gather (dma_gather, int16 idx -> edges split by table row <
    32768), attention weights w = exp(leaky_relu(as[src] + ad[dst])) on-chip
    (ad gathered from a core-local bf16 table), aggregation via one-hot
    slot-matmul on the PE: B.T @ (w * [1|1|h]) giving per-node numerators and
    (via the duplicated ones columns) denominators in one pass.
  - block results land contiguously in DRAM ("seq" buffers, bf16); the
    finalize pass gathers each node's L/H partial rows, adds, normalizes.

DVE ops use 16-bit dtypes with pair-replicated scalars so the per-element
broadcast multiplies hit the DVE 2x packed mode.
"""
import numpy as np
import ml_dtypes

BF16 = ml_dtypes.bfloat16

# ---- problem constants (hardcoded per contract) ----
N = 50000
F_IN = 128
HID = 64
H0 = 4
N_CORES = 8
NPC = N // N_CORES            # 6250
NLOCPAD = 6272                # 49*128: padded rows per core (table row space)
CHA = 3200                    # chunk-A rows per core (25 tiles)
CHB = 3072                    # chunk-B rows per core (24 tiles)
NRA = CHA * N_CORES           # 25600 rows in table chunk A (< 2^15 for int16)
NRB = CHB * N_CORES           # 24576 rows in table chunk B
SLOTS = 64
TRASH = SLOTS - 1             # 63
GEO = {"A": dict(BLK=512, SUB=4, SBB=16),   # 8192 edges / superblock
       "B": dict(BLK=512, SUB=4, SBB=16)}
SBE = 8192
SUBMAX = 6
RU0 = 384                     # bf16 units per table0 row (768B); 268 used
RU1 = 128                     # table1 row units (256B); 68 used
ADB = 128                     # ad table row bf16 units (256B)
OW0 = 264                     # seq0 used cols (bf16), row stride RU0
OW1 = 68                      # seq1 used cols (bf16), row stride RU1
HW0 = 66                      # layer-0 per-head block: [1,1,h*64]
NT_C = NLOCPAD // 128         # 49
LAST_C = NPC - 48 * 128       # 106
NCHUNK = 7                    # finalize gather chunks of 1024 nodes

_prog_cache = {}


def _wrap16(idx, pad_to=None):
    """ucode idx layout: idx i at [i%16, i//16], replicated to 128 partitions."""
    idx = np.asarray(idx, np.int16)
    if pad_to is not None and len(idx) < pad_to:
        idx = np.concatenate([idx, np.zeros(pad_to - len(idx), np.int16)])
    n = len(idx)
    a = idx.reshape(n // 16, 16).T.copy()
    return np.tile(a, (8, 1))


def _pack_half(ss, dd, geo):
    """Greedy-pack edges (dst-sorted local) into BLK-edge / 63-slot blocks."""
    BLK = geo["BLK"]
    blocks = []
    if len(ss):
        uniq, starts = np.unique(dd, return_index=True)
        ends = np.append(starts[1:], len(dd))
        cur_s, cur_nodes, cur_slot = [], [], []
        for nd, st, en in zip(uniq, starts, ends):
            deg = en - st
            if len(cur_s) + deg > BLK or len(cur_nodes) >= TRASH:
                blocks.append((cur_s, cur_slot, cur_nodes))
                cur_s, cur_nodes, cur_slot = [], [], []
            sl = len(cur_nodes)
            cur_nodes.append(nd)
            cur_s.extend(ss[st:en])
            cur_slot.extend([sl] * deg)
        if cur_s:
            blocks.append((cur_s, cur_slot, cur_nodes))
    return blocks


def _pack_core(src, dst, core):
    lo, hi = core * NPC, (core + 1) * NPC
    m = (dst >= lo) & (dst < hi)
    s = src[m]
    d_loc = dst[m] - lo
    order = np.argsort(d_loc, kind="stable")
    s, d_loc = s[order], d_loc[order]
    sc, so = s // NPC, s % NPC          # owning core, local offset
    in_a = so < CHA
    out = {}
    for half, sel in (("A", in_a), ("B", ~in_a)):
        ss = np.where(in_a, sc * CHA + so, sc * CHB + (so - CHA))[sel]
        out[half] = _pack_half(ss, d_loc[sel], GEO[half])
    return out


def _streams_for_half(blocks, n_sb_target, geo):
    BLK, SBB = geo["BLK"], geo["SBB"]
    nbt = n_sb_target * SBB
    src_b = np.zeros((nbt, BLK), np.int32)
    slot_b = np.full((nbt, BLK), TRASH, np.int32)
    dloc_b = np.zeros((nbt, BLK), np.int32)
    rowpos = np.full(NLOCPAD + 1024, nbt * SLOTS, np.int32)  # default: zero row
    for j, (s_, sl_, nds) in enumerate(blocks):
        k = len(s_)
        src_b[j, :k] = s_
        slot_b[j, :k] = sl_
        nda = np.asarray(nds, np.int32)
        dloc_b[j, :k] = nda[np.asarray(sl_, np.int32)]
        rowpos[nda] = j * SLOTS + np.arange(len(nds), dtype=np.int32)
    src_sb = src_b.reshape(n_sb_target, SBB * BLK)
    dloc_sb = dloc_b.reshape(n_sb_target, SBB * BLK)
    hidx = np.stack([_wrap16(r.astype(np.int16)) for r in src_sb])
    adidx = np.stack([_wrap16(r.astype(np.int16)) for r in dloc_sb])
    # paired bf16 slot stream: [n_sb, 128, SBB*SUB, 2]
    slotb = (slot_b.reshape(n_sb_target, SBB * (BLK // 128), 128)
             .transpose(0, 2, 1))                 # [n_sb, 128, SBB*SUB]
    slotb2 = np.repeat(slotb.astype(BF16), 2, axis=2).reshape(n_sb_target, 128, -1)
    # finalize gather idx: chunks of 1024 node ids
    rows = rowpos[:NCHUNK * 1024].astype(np.int16)
    rowchunks = np.stack([_wrap16(rows[u * 1024:(u + 1) * 1024])
                          for u in range(NCHUNK)])
    return dict(hidx=hidx.astype(np.int16), adidx=adidx.astype(np.int16),
                slotb2=slotb2, rowchunks=rowchunks.astype(np.int16))


def host_prepare(inputs):
    x = np.ascontiguousarray(np.asarray(inputs["x"], np.float32))
    ei = np.asarray(inputs["edge_index"], np.int32)
    W0 = np.asarray(inputs["W0"], np.float32)
    as0 = np.asarray(inputs["att_src0"], np.float32)
    ad0 = np.asarray(inputs["att_dst0"], np.float32)
    b0 = np.asarray(inputs["bias0"], np.float32)
    W1 = np.asarray(inputs["W1"], np.float32)
    as1 = np.asarray(inputs["att_src1"], np.float32)
    ad1 = np.asarray(inputs["att_dst1"], np.float32)
    b1 = np.asarray(inputs["bias1"], np.float32)

    A_s0 = np.einsum("ihc,hc->ih", W0.reshape(F_IN, H0, HID), as0).astype(np.float32)
    A_d0 = np.einsum("ihc,hc->ih", W0.reshape(F_IN, H0, HID), ad0).astype(np.float32)
    A_sd0 = np.concatenate([A_s0, A_d0], axis=1)  # [F_IN, 8]
    A_sd1 = np.stack([
        np.einsum("ihc,hc->ih", W1.reshape(H0 * HID, 1, HID), as1)[:, 0],
        np.einsum("ihc,hc->ih", W1.reshape(H0 * HID, 1, HID), ad1)[:, 0],
    ], axis=1).astype(np.float32)

    loop = np.arange(N, dtype=np.int32)
    src = np.concatenate([ei[0], loop])
    dst = np.concatenate([ei[1], loop])

    packs = [_pack_core(src, dst, c) for c in range(N_CORES)]
    n_sb = {h: max((len(p[h]) + GEO[h]["SBB"] - 1) // GEO[h]["SBB"] for p in packs)
            for h in ("A", "B")}

    common = {
        "W0b": W0.astype(BF16),
        "A_sd0": A_sd0.astype(BF16),
        "W1b": W1.astype(BF16),
        "A_sd1b": A_sd1.astype(BF16),
        "bias0t": np.tile(b0[None, :], (128, 1)).astype(BF16),
        "bias1t": np.tile(b1[None, :], (128, 1)).astype(BF16),
        "iota64": np.tile(np.arange(SLOTS, dtype=np.float32), (128, SUBMAX, 1))
                    .reshape(128, SUBMAX * SLOTS).astype(BF16),
        "ones_bf": np.ones((128, 1), BF16),
        "identb": np.eye(128, dtype=np.float32).astype(BF16),
    }
    in_maps = []
    for c in range(N_CORES):
        d = dict(common)
        xl = np.zeros((NLOCPAD, F_IN), np.float32)
        xl[:NPC] = x[c * NPC:(c + 1) * NPC]
        d["xT"] = np.ascontiguousarray(xl.T).astype(BF16)  # [F_IN, NLOCPAD]
        for half in ("A", "B"):
            st = _streams_for_half(packs[c][half], n_sb[half], GEO[half])
            for k, v in st.items():
                d[f"{k}_{half}"] = v.reshape(-1, v.shape[-1])
        in_maps.append(d)
    return in_maps, n_sb


# ----------------------------------------------------------------------------
# bass program
# ----------------------------------------------------------------------------

def build_program(n_sb, phases="full"):
    import concourse.bass as bass
    import concourse.bacc as bacc
    import concourse.tile as tile
    import concourse.mybir as mybir
    dt = mybir.dt

    import os as _os
    GCH = int(_os.environ.get("KGAT_GCH", "1024"))
    nc = bacc.Bacc("TRN2", target_bir_lowering=False, debug=False,
                   enable_asserts=False, num_devices=N_CORES,
                   num_swdge_queues=4,
                   dynamic_dma_scratch_size=16384 * (GCH // 1024))

    _gq = [0]

    def gather(out_ap, in_ap, idxs_ap, num_idxs, elem_size):
        # dma_gather corrupts above the SWDGE ring capacity; chunk at GCH.
        done = 0
        while done < num_idxs:
            ch = min(GCH, num_idxs - done)
            assert ch % 128 == 0
            nc.gpsimd.dma_gather(
                out_ap=out_ap[:, done // 128:(done + ch) // 128, :],
                in_ap=in_ap,
                idxs_ap=idxs_ap[:, done // 16:(done + ch) // 16],
                num_idxs=ch, num_idxs_reg=ch, elem_size=elem_size,
                queue_num=_gq[0] % 4)
            _gq[0] += 1
            done += ch

    def inp(name, shape, dtype):
        return nc.dram_tensor(name, shape, dtype, kind="ExternalInput").ap()

    xT = inp("xT", [F_IN, NLOCPAD], dt.bfloat16)
    W0b = inp("W0b", [F_IN, H0 * HID], dt.bfloat16)
    A_sd0 = inp("A_sd0", [F_IN, 2 * H0], dt.bfloat16)
    W1b = inp("W1b", [H0 * HID, HID], dt.bfloat16)
    A_sd1b = inp("A_sd1b", [H0 * HID, 2], dt.bfloat16)
    bias0t = inp("bias0t", [128, H0 * HID], dt.bfloat16)
    bias1t = inp("bias1t", [128, HID], dt.bfloat16)
    iota64 = inp("iota64", [128, SUBMAX * SLOTS], dt.bfloat16)
    ones_bf = inp("ones_bf", [128, 1], dt.bfloat16)
    identb = inp("identb", [128, 128], dt.bfloat16)
    streams = {}
    nrows = {}
    for half in ("A", "B"):
        ns, SBB, SUBh = n_sb[half], GEO[half]["SBB"], GEO[half]["SUB"]
        streams[half] = dict(
            hidx=inp(f"hidx_{half}", [ns * 128, SBE // 16], dt.int16),
            adidx=inp(f"adidx_{half}", [ns * 128, SBE // 16], dt.int16),
            slotb2=inp(f"slotb2_{half}", [ns * 128, SBB * SUBh * 2], dt.bfloat16),
            rowchunks=inp(f"rowchunks_{half}", [NCHUNK * 128, 64], dt.int16),
        )
        nrows[half] = ns * SBB * SLOTS + 128   # + zero block
    out_f = nc.dram_tensor("out", [NPC, HID], dt.float32, kind="ExternalOutput").ap()

    CH = {"A": CHA, "B": CHB}
    NR = {"A": NRA, "B": NRB}
    t0loc = {h: nc.dram_tensor(f"t0loc{h}", [CH[h], RU0], dt.bfloat16).ap()
             for h in ("A", "B")}
    tab0 = {h: nc.dram_tensor(f"tab0{h}", [NR[h], RU0], dt.bfloat16,
                              addr_space="Shared").ap() for h in ("A", "B")}
    ad0_loc = nc.dram_tensor("ad0_loc", [NLOCPAD, ADB], dt.bfloat16).ap()
    seq0 = {h: nc.dram_tensor(f"seq0{h}", [nrows[h], RU0], dt.bfloat16).ap()
            for h in ("A", "B")}
    t1loc = {h: nc.dram_tensor(f"t1loc{h}", [CH[h], RU1], dt.bfloat16).ap()
             for h in ("A", "B")}
    tab1 = {h: nc.dram_tensor(f"tab1{h}", [NR[h], RU1], dt.bfloat16,
                              addr_space="Shared").ap() for h in ("A", "B")}
    ad1_loc = nc.dram_tensor("ad1_loc", [NLOCPAD, ADB], dt.bfloat16).ap()
    seq1 = {h: nc.dram_tensor(f"seq1{h}", [nrows[h], RU1], dt.bfloat16).ap()
            for h in ("A", "B")}
    NT_A = CHA // 128   # 25 tiles in chunk A

    AF = mybir.ActivationFunctionType
    OP = mybir.AluOpType

    def pair_bcast(ap_pk2, outer, inner):
        """[128, outer, 2] -> broadcast AP [128, outer, inner, 2]."""
        return (ap_pk2.rearrange("p a b -> p a b ()")
                .rearrange("p a b u -> p a u b")
                .to_broadcast([128, outer, inner, 2]))

    with tile.TileContext(nc) as tc:
        with (
            tc.tile_pool(name="const", bufs=1) as cpool,
            tc.tile_pool(name="sbuf", bufs=3) as pool,
            tc.tile_pool(name="gath", bufs=2) as gpool,
            tc.tile_pool(name="psum", bufs=2, space="PSUM") as psum,
            tc.tile_pool(name="psB", bufs=2, space="PSUM") as psumB,
        ):
            identt = cpool.tile([128, 128], dt.bfloat16)
            nc.sync.dma_start(out=identt[:], in_=identb[:])
            W0bt = cpool.tile([128, H0 * HID], dt.bfloat16)
            nc.sync.dma_start(out=W0bt[:], in_=W0b[:])
            A_sd0t = cpool.tile([128, 2 * H0], dt.bfloat16)
            nc.sync.dma_start(out=A_sd0t[:], in_=A_sd0[:])
            W1bt = cpool.tile([128, 2, HID], dt.bfloat16)
            nc.sync.dma_start(out=W1bt[:], in_=W1b[:].rearrange("(a p) d -> p a d", p=128))
            A_sd1t = cpool.tile([128, 2, 2], dt.bfloat16)
            nc.sync.dma_start(out=A_sd1t[:], in_=A_sd1b[:].rearrange("(a p) d -> p a d", p=128))
            bias0tt = cpool.tile([128, H0 * HID], dt.bfloat16)
            nc.sync.dma_start(out=bias0tt[:], in_=bias0t[:])
            bias1tt = cpool.tile([128, HID], dt.bfloat16)
            nc.sync.dma_start(out=bias1tt[:], in_=bias1t[:])
            iota64t = cpool.tile([128, SUBMAX, SLOTS], dt.bfloat16)
            nc.sync.dma_start(out=iota64t[:],
                              in_=iota64[:].rearrange("p (a b) -> p a b", a=SUBMAX))
            ones_bft = cpool.tile([128, 1], dt.bfloat16)
            nc.sync.dma_start(out=ones_bft[:], in_=ones_bf[:])
            zeroB = cpool.tile([128, RU0], dt.bfloat16)
            nc.vector.memset(zeroB[:], 0.0)

            # zero rows at tail of each seq buffer
            for h in ("A", "B"):
                nc.sync.dma_start(out=seq0[h][nrows[h] - 128:, :], in_=zeroB[:])
                nc.sync.dma_start(out=seq1[h][nrows[h] - 128:, :], in_=zeroB[:, :RU1])

            # ---- phase A: local table0 slice + ad0 table (sharded) ----
            def phase_a_tile(t):
                r0 = t * 128
                xt = pool.tile([128, 128], dt.bfloat16, tag="xt")
                nc.sync.dma_start(out=xt[:], in_=xT[:, r0:r0 + 128])
                psH = psum.tile([128, H0 * HID], dt.float32, tag="psH")
                nc.tensor.matmul(out=psH[:], lhsT=xt[:], rhs=W0bt[:], start=True, stop=True)
                psA = psum.tile([128, 2 * H0], dt.float32, tag="psA")
                nc.tensor.matmul(out=psA[:], lhsT=xt[:], rhs=A_sd0t[:], start=True, stop=True)
                stag = pool.tile([128, RU0], dt.bfloat16, tag="stag")
                nc.vector.tensor_copy(out=stag[:, 0:H0], in_=psA[:, 0:H0])
                sv = stag[:, H0:H0 + H0 * HW0].rearrange("p (h u) -> p h u", h=H0)
                nc.vector.tensor_copy(
                    out=sv[:, :, 0:2],
                    in_=ones_bft[:].rearrange("p u -> p u ()").to_broadcast([128, H0, 2]))
                nc.vector.tensor_copy(
                    out=sv[:, :, 2:HW0],
                    in_=psH[:].rearrange("p (h u) -> p h u", h=H0))
                if t < NT_A:
                    nc.sync.dma_start(out=t0loc["A"][r0:r0 + 128, :], in_=stag[:])
                else:
                    nc.sync.dma_start(out=t0loc["B"][r0 - CHA:r0 - CHA + 128, :],
                                      in_=stag[:])
                adst = pool.tile([128, ADB], dt.bfloat16, tag="adst")
                nc.vector.tensor_copy(out=adst[:, 0:H0], in_=psA[:, H0:2 * H0])
                nc.sync.dma_start(out=ad0_loc[r0:r0 + 128, :], in_=adst[:])

            def ag(src_ap, dst_ap):
                nc.gpsimd.collective_compute(
                    "AllGather", OP.bypass, replica_groups=[list(range(N_CORES))],
                    ins=[src_ap.opt()], outs=[dst_ap.opt()])

            if phases != "empty":
                for t in range(NT_A):
                    phase_a_tile(t)
                if phases in ("B", "C", "G", "full"):
                    ag(t0loc["A"], tab0["A"])
                for t in range(NT_A, NT_C):
                    phase_a_tile(t)
                if phases in ("B", "C", "G", "full"):
                    ag(t0loc["B"], tab0["B"])

            # ---- edge phase ----
            def edge_phase(layer):
                if layer == 0:
                    tabs, ad_loc, seqT, ru, nheads = tab0, ad0_loc, seq0, RU0, H0
                    as_u, hw, mo, ow = H0, HW0, H0, OW0
                else:
                    tabs, ad_loc, seqT, ru, nheads = tab1, ad1_loc, seq1, RU1, 1
                    as_u, hw, mo, ow = 1, OW1, 0, OW1
                import os as _os
                _halves = _os.environ.get("KGAT_HALVES", "AB")
                _maxsb = int(_os.environ.get("KGAT_MAXSB", "9999"))
                for half in [h for h in ("A", "B") if h in _halves]:
                    SUBh, SBB = GEO[half]["SUB"], GEO[half]["SBB"]
                    tab = tabs[half]
                    st = streams[half]
                    for s in range(min(n_sb[half], _maxsb)):
                        hix = pool.tile([128, SBE // 16], dt.int16, tag="hix")
                        nc.sync.dma_start(out=hix[:], in_=st["hidx"][s * 128:(s + 1) * 128, :])
                        aix = pool.tile([128, SBE // 16], dt.int16, tag="aix")
                        nc.sync.dma_start(out=aix[:], in_=st["adidx"][s * 128:(s + 1) * 128, :])
                        slt = pool.tile([128, SBB * SUBh, 2], dt.bfloat16, tag="slt")
                        nc.sync.dma_start(
                            out=slt[:],
                            in_=st["slotb2"][s * 128:(s + 1) * 128, :]
                                .rearrange("p (a b) -> p a b", b=2))

                        hg = gpool.tile([128, SBE // 128, ru], dt.bfloat16, tag="hg")
                        gather(hg[:], tab, hix[:], SBE, ru)
                        adg = gpool.tile([128, SBE // 128, ADB], dt.bfloat16, tag="adg")
                        gather(adg[:], ad_loc[:], aix[:], SBE, ADB)

                        stage = pool.tile([128, SBB // 2, ru], dt.bfloat16, tag="stage")
                        for b in range(SBB):
                            g0 = b * SUBh
                            m = 64 * (b % 2)
                            if b % 2 == 0:
                                ps = psumB.tile([128, ow], dt.float32, tag="psB")
                            # attention logits + weights for this block
                            ev = pool.tile([128, SUBh, nheads], dt.bfloat16, tag="ev")
                            nc.vector.tensor_tensor(
                                out=ev[:], in0=hg[:, g0:g0 + SUBh, 0:nheads],
                                in1=adg[:, g0:g0 + SUBh, 0:nheads], op=OP.add)
                            tv = pool.tile([128, SUBh, nheads], dt.bfloat16, tag="tv")
                            nc.vector.tensor_scalar_mul(out=tv[:], in0=ev[:], scalar1=0.2)
                            nc.vector.tensor_tensor(out=tv[:], in0=tv[:], in1=ev[:], op=OP.max)
                            wv = pool.tile([128, SUBh, nheads], dt.bfloat16, tag="wv")
                            nc.scalar.activation(out=wv[:], in_=tv[:], func=AF.Exp)
                            wv2 = pool.tile([128, SUBh, nheads, 2], dt.bfloat16, tag="wv2")
                            nc.vector.tensor_copy(
                                out=wv2[:],
                                in_=wv[:].rearrange("p a h -> p a h ()")
                                    .to_broadcast([128, SUBh, nheads, 2]))
                            B8 = pool.tile([128, SUBh, SLOTS], dt.bfloat16, tag="B8")
                            nc.vector.tensor_tensor(
                                out=B8[:].rearrange("p a (c b) -> p a c b", b=2),
                                in0=pair_bcast(slt[:, g0:g0 + SUBh, :], SUBh, SLOTS // 2),
                                in1=iota64t[:, :SUBh, :].rearrange("p a (c b) -> p a c b", b=2),
                                op=OP.is_equal)
                            rhs = pool.tile([128, SUBh, nheads * hw], dt.bfloat16,
                                            tag="rhs")
                            for hh in range(nheads):
                                o = mo + hw * hh
                                nc.vector.tensor_tensor(
                                    out=rhs[:, :, hw * hh:hw * hh + hw]
                                        .rearrange("p a (c b) -> p a c b", b=2),
                                    in0=hg[:, g0:g0 + SUBh, o:o + hw]
                                        .rearrange("p a (c b) -> p a c b", b=2),
                                    in1=pair_bcast(wv2[:, :, hh, :], SUBh, hw // 2),
                                    op=OP.mult)
                            for k in range(SUBh):
                                nc.tensor.matmul(
                                    out=ps[m:m + 64, 0:nheads * hw],
                                    lhsT=B8[:, k, :], rhs=rhs[:, k, :],
                                    start=(k == 0), stop=(k == SUBh - 1))
                            if b % 2 == 1:
                                c = b // 2
                                nc.vector.tensor_copy(out=stage[:, c, 0:ow], in_=ps[:, 0:ow])
                        r0 = s * SBB * SLOTS
                        nc.sync.dma_start(
                            out=seqT[half][r0:r0 + SBB * SLOTS, :]
                                .rearrange("(c p) u -> p c u", p=128),
                            in_=stage[:])

            if phases in ("B", "C", "G", "full"):
                edge_phase(0)

            # ---- phase C: finalize layer-0, build table1 local slice ----
            for u in (range(NCHUNK) if phases in ("C", "G", "full") else []):
                gL = gpool.tile([128, 8, RU0], dt.bfloat16, tag="hg")
                gH = gpool.tile([128, 8, RU0], dt.bfloat16, tag="adg")
                for h, g in (("A", gL), ("B", gH)):
                    rix = pool.tile([128, 64], dt.int16, tag="rix")
                    nc.sync.dma_start(out=rix[:],
                                      in_=streams[h]["rowchunks"][u * 128:(u + 1) * 128, :])
                    gather(g[:], seq0[h][:], rix[:], 1024, RU0)
                for tt in range(8):
                    t = u * 8 + tt
                    if t >= NT_C:
                        break
                    r0 = t * 128
                    cnt = 128 if t < NT_C - 1 else LAST_C
                    o = pool.tile([128, OW0], dt.bfloat16, tag="oC")
                    nc.vector.tensor_tensor(out=o[:], in0=gL[:, tt, 0:OW0],
                                            in1=gH[:, tt, 0:OW0], op=OP.add)
                    ov = o[:].rearrange("p (h u) -> p h u", h=H0)
                    rec = pool.tile([128, H0], dt.float32, tag="rec")
                    nc.vector.reciprocal(out=rec[:],
                                         in_=ov[:, :, 0:1].rearrange("p h u -> p (h u)"))
                    rec2 = pool.tile([128, H0, 2], dt.bfloat16, tag="rec2")
                    nc.vector.tensor_copy(
                        out=rec2[:],
                        in_=rec[:].rearrange("p h -> p h ()").to_broadcast([128, H0, 2]))
                    z = pool.tile([128, H0, HID], dt.bfloat16, tag="z")
                    nc.vector.tensor_tensor(
                        out=z[:].rearrange("p h (c b) -> p h c b", b=2),
                        in0=ov[:, :, 2:HW0].rearrange("p h (c b) -> p h c b", b=2),
                        in1=pair_bcast(rec2[:], H0, HID // 2),
                        op=OP.mult)
                    zf = z[:].rearrange("p h u -> p (h u)")
                    nc.vector.tensor_tensor(out=zf, in0=zf, in1=bias0tt[:], op=OP.add)
                    zm = pool.tile([128, H0 * HID], dt.bfloat16, tag="zm")
                    nc.vector.tensor_scalar_min(out=zm[:], in0=zf, scalar1=0.0)
                    qe = pool.tile([128, H0 * HID], dt.bfloat16, tag="qe")
                    nc.scalar.activation(out=qe[:], in_=zm[:], func=AF.Exp)
                    nc.vector.tensor_scalar(out=zf, in0=zf, scalar1=0.0, scalar2=-1.0,
                                            op0=OP.max, op1=OP.add)
                    nc.vector.tensor_tensor(out=zf, in0=zf, in1=qe[:], op=OP.add)
                    psH1 = psum.tile([128, HID], dt.float32, tag="psH")
                    psA1 = psum.tile([128, 2], dt.float32, tag="psA")
                    for ch in range(2):
                        psT = psum.tile([128, 128], dt.bfloat16, tag="psT")
                        nc.tensor.transpose(out=psT[:],
                                            in_=zf[:, 128 * ch:128 * ch + 128],
                                            identity=identt[:])
                        zTb = pool.tile([128, 128], dt.bfloat16, tag="zTb")
                        nc.vector.tensor_copy(out=zTb[:], in_=psT[:])
                        nc.tensor.matmul(out=psH1[:], lhsT=zTb[:], rhs=W1bt[:, ch, :],
                                         start=(ch == 0), stop=(ch == 1))
                        nc.tensor.matmul(out=psA1[:], lhsT=zTb[:], rhs=A_sd1t[:, ch, :],
                                         start=(ch == 0), stop=(ch == 1))
                    t1s = pool.tile([128, RU1], dt.bfloat16, tag="t1s")
                    nc.vector.tensor_copy(
                        out=t1s[:, 0:2],
                        in_=psA1[:, 0:1].to_broadcast([128, 2]))
                    nc.vector.tensor_copy(
                        out=t1s[:, 2:4],
                        in_=ones_bft[:].to_broadcast([128, 2]))
                    nc.vector.tensor_copy(out=t1s[:, 4:4 + HID], in_=psH1[:])
                    if t < NT_A:
                        nc.sync.dma_start(out=t1loc["A"][r0:r0 + cnt, :],
                                          in_=t1s[0:cnt, :])
                    else:
                        nc.sync.dma_start(out=t1loc["B"][r0 - CHA:r0 - CHA + cnt, :],
                                          in_=t1s[0:cnt, :])
                    a1s = pool.tile([128, ADB], dt.bfloat16, tag="adst")
                    nc.vector.tensor_copy(out=a1s[:, 0:1], in_=psA1[:, 1:2])
                    nc.sync.dma_start(out=ad1_loc[r0:r0 + 128, :], in_=a1s[:])
                    if t == NT_A - 1 and phases in ("G", "full"):
                        ag(t1loc["A"], tab1["A"])   # overlap with B-chunk finalize

            # ---- AllGather table1 (B chunk) ----
            if phases in ("G", "full"):
                ag(t1loc["B"], tab1["B"])

            if phases == "full":
                edge_phase(1)

            # ---- phase E: finalize layer-1 ----
            for u in (range(NCHUNK) if phases == "full" else []):
                gL = gpool.tile([128, 8, RU1], dt.bfloat16, tag="hg")
                gH = gpool.tile([128, 8, RU1], dt.bfloat16, tag="adg")
                for h, g in (("A", gL), ("B", gH)):
                    rix = pool.tile([128, 64], dt.int16, tag="rix")
                    nc.sync.dma_start(out=rix[:],
                                      in_=streams[h]["rowchunks"][u * 128:(u + 1) * 128, :])
                    gather(g[:], seq1[h][:], rix[:], 1024, RU1)
                for tt in range(8):
                    t = u * 8 + tt
                    if t >= NT_C:
                        break
                    r0 = t * 128
                    cnt = 128 if t < NT_C - 1 else LAST_C
                    o = pool.tile([128, OW1], dt.bfloat16, tag="o1")
                    nc.vector.tensor_tensor(out=o[:], in0=gL[:, tt, 0:OW1],
                                            in1=gH[:, tt, 0:OW1], op=OP.add)
                    rec = pool.tile([128, 1], dt.float32, tag="rec1")
                    nc.vector.reciprocal(out=rec[:], in_=o[:, 2:3])
                    rec2 = pool.tile([128, 2], dt.bfloat16, tag="rec12")
                    nc.vector.tensor_copy(out=rec2[:], in_=rec[:].to_broadcast([128, 2]))
                    res = pool.tile([128, HID], dt.bfloat16, tag="res")
                    nc.vector.tensor_tensor(
                        out=res[:].rearrange("p (c b) -> p c b", b=2),
                        in0=o[:, 4:4 + HID].rearrange("p (c b) -> p c b", b=2),
                        in1=rec2[:].rearrange("p b -> p b ()").rearrange("p b u -> p u b")
                            .to_broadcast([128, HID // 2, 2]),
                        op=OP.mult)
                    resf = pool.tile([128, HID], dt.float32, tag="resf")
                    nc.vector.tensor_tensor(out=resf[:], in0=res[:], in1=bias1tt[:], op=OP.add)
                    nc.sync.dma_start(out=out_f[r0:r0 + cnt, :], in_=resf[0:cnt, :])

            if phases != "full":
                for t in range(NT_C):
                    r0 = t * 128
                    cnt = 128 if t < NT_C - 1 else LAST_C
                    zf32 = cpool.tile([128, HID], dt.float32)
                    nc.vector.memset(zf32[:], 0.0)
                    nc.sync.dma_start(out=out_f[r0:r0 + cnt, :], in_=zf32[0:cnt, :])

    nc.compile()
    return nc


def kernel(**inputs):
    import os
    from concourse import bass_utils
    in_maps, n_sb = host_prepare(inputs)
    phases = os.environ.get("KGAT_PHASES", "full")
    key = (n_sb["A"], n_sb["B"], phases)
    if key not in _prog_cache:
        _prog_cache[key] = build_program(n_sb, phases)
    nc = _prog_cache[key]
    res = bass_utils.run_bass_kernel_spmd(nc, in_maps, core_ids=list(range(N_CORES)))
    out = np.concatenate([np.asarray(res.results[c]["out"]) for c in range(N_CORES)], axis=0)
    return out.astype(np.float32)


# revision 34
# speedup vs baseline: 1.0530x; 1.0530x over previous
"""Trainium2 Bass kernel for 2-layer GAT (nn_GAT_47957604827269).

Strategy: partition nodes across 8 cores by dst range. Per layer:
  - per-core table slice build (local x^T tiles -> PE matmuls), AllGather to
    a full per-node feature table in DRAM: row = [as | (1,1,h)*H] in bf16,
    256B-aligned rows for the dma_gather ucode op.
  - per-edge gather (dma_gather, int16 idx -> edges split by table row <
    32768), attention weights w = exp(leaky_relu(as[src] + ad[dst])) on-chip
    (ad gathered from a core-local bf16 table), aggregation via one-hot
    slot-matmul on the PE: B.T @ (w * [1|1|h]) giving per-node numerators and
    (via the duplicated ones columns) denominators in one pass.
  - block results land contiguously in DRAM ("seq" buffers, bf16); the
    finalize pass gathers each node's L/H partial rows, adds, normalizes.

DVE ops use 16-bit dtypes with pair-replicated scalars so the per-element
broadcast multiplies hit the DVE 2x packed mode.
"""
import numpy as np
import ml_dtypes

BF16 = ml_dtypes.bfloat16

# ---- problem constants (hardcoded per contract) ----
N = 50000
F_IN = 128
HID = 64
H0 = 4
N_CORES = 8
NPC = N // N_CORES            # 6250
NLOCPAD = 6272                # 49*128: padded rows per core (table row space)
CHA = 3200                    # chunk-A rows per core (25 tiles)
CHB = 3072                    # chunk-B rows per core (24 tiles)
NRA = CHA * N_CORES           # 25600 rows in table chunk A (< 2^15 for int16)
NRB = CHB * N_CORES           # 24576 rows in table chunk B
SLOTS = 64
TRASH = SLOTS - 1             # 63
GEO = {"A": dict(BLK=512, SUB=4, SBB=8),    # 4096 edges / superblock
       "B": dict(BLK=512, SUB=4, SBB=8)}
SBE = 4096
SUBMAX = 6
RU0 = 384                     # bf16 units per table0 row (768B); 268 used
RU1 = 128                     # table1 row units (256B); 68 used
ADB = 128                     # ad table row bf16 units (256B)
OW0 = 264                     # seq0 used cols (bf16), row stride RU0
OW1 = 68                      # seq1 used cols (bf16), row stride RU1
HW0 = 66                      # layer-0 per-head block: [1,1,h*64]
NT_C = NLOCPAD // 128         # 49
LAST_C = NPC - 48 * 128       # 106
NCHUNK = 7                    # finalize gather chunks of 1024 nodes

_prog_cache = {}


def _wrap16(idx, pad_to=None):
    """ucode idx layout: idx i at [i%16, i//16], replicated to 128 partitions."""
    idx = np.asarray(idx, np.int16)
    if pad_to is not None and len(idx) < pad_to:
        idx = np.concatenate([idx, np.zeros(pad_to - len(idx), np.int16)])
    n = len(idx)
    a = idx.reshape(n // 16, 16).T.copy()
    return np.tile(a, (8, 1))


def _pack_half(ss, dd, geo):
    """Greedy-pack edges (dst-sorted local) into BLK-edge / 63-slot blocks."""
    BLK = geo["BLK"]
    blocks = []
    if len(ss):
        uniq, starts = np.unique(dd, return_index=True)
        ends = np.append(starts[1:], len(dd))
        cur_s, cur_nodes, cur_slot = [], [], []
        for nd, st, en in zip(uniq, starts, ends):
            deg = en - st
            if len(cur_s) + deg > BLK or len(cur_nodes) >= TRASH:
                blocks.append((cur_s, cur_slot, cur_nodes))
                cur_s, cur_nodes, cur_slot = [], [], []
            sl = len(cur_nodes)
            cur_nodes.append(nd)
            cur_s.extend(ss[st:en])
            cur_slot.extend([sl] * deg)
        if cur_s:
            blocks.append((cur_s, cur_slot, cur_nodes))
    return blocks


def _pack_core(src, dst, core):
    lo, hi = core * NPC, (core + 1) * NPC
    m = (dst >= lo) & (dst < hi)
    s = src[m]
    d_loc = dst[m] - lo
    order = np.argsort(d_loc, kind="stable")
    s, d_loc = s[order], d_loc[order]
    sc, so = s // NPC, s % NPC          # owning core, local offset
    in_a = so < CHA
    out = {}
    for half, sel in (("A", in_a), ("B", ~in_a)):
        ss = np.where(in_a, sc * CHA + so, sc * CHB + (so - CHA))[sel]
        out[half] = _pack_half(ss, d_loc[sel], GEO[half])
    return out


def _streams_for_half(blocks, n_sb_target, geo):
    BLK, SBB = geo["BLK"], geo["SBB"]
    nbt = n_sb_target * SBB
    src_b = np.zeros((nbt, BLK), np.int32)
    slot_b = np.full((nbt, BLK), TRASH, np.int32)
    dloc_b = np.zeros((nbt, BLK), np.int32)
    rowpos = np.full(NLOCPAD + 1024, nbt * SLOTS, np.int32)  # default: zero row
    for j, (s_, sl_, nds) in enumerate(blocks):
        k = len(s_)
        src_b[j, :k] = s_
        slot_b[j, :k] = sl_
        nda = np.asarray(nds, np.int32)
        dloc_b[j, :k] = nda[np.asarray(sl_, np.int32)]
        rowpos[nda] = j * SLOTS + np.arange(len(nds), dtype=np.int32)
    src_sb = src_b.reshape(n_sb_target, SBB * BLK)
    dloc_sb = dloc_b.reshape(n_sb_target, SBB * BLK)
    hidx = np.stack([_wrap16(r.astype(np.int16)) for r in src_sb])
    adidx = np.stack([_wrap16(r.astype(np.int16)) for r in dloc_sb])
    # paired bf16 slot stream: [n_sb, 128, SBB*SUB, 2]
    slotb = (slot_b.reshape(n_sb_target, SBB * (BLK // 128), 128)
             .transpose(0, 2, 1))                 # [n_sb, 128, SBB*SUB]
    slotb2 = np.repeat(slotb.astype(BF16), 2, axis=2).reshape(n_sb_target, 128, -1)
    # finalize gather idx: chunks of 1024 node ids
    rows = rowpos[:NCHUNK * 1024].astype(np.int16)
    rowchunks = np.stack([_wrap16(rows[u * 1024:(u + 1) * 1024])
                          for u in range(NCHUNK)])
    return dict(hidx=hidx.astype(np.int16), adidx=adidx.astype(np.int16),
                slotb2=slotb2, rowchunks=rowchunks.astype(np.int16))


def host_prepare(inputs):
    x = np.ascontiguousarray(np.asarray(inputs["x"], np.float32))
    ei = np.asarray(inputs["edge_index"], np.int32)
    W0 = np.asarray(inputs["W0"], np.float32)
    as0 = np.asarray(inputs["att_src0"], np.float32)
    ad0 = np.asarray(inputs["att_dst0"], np.float32)
    b0 = np.asarray(inputs["bias0"], np.float32)
    W1 = np.asarray(inputs["W1"], np.float32)
    as1 = np.asarray(inputs["att_src1"], np.float32)
    ad1 = np.asarray(inputs["att_dst1"], np.float32)
    b1 = np.asarray(inputs["bias1"], np.float32)

    A_s0 = np.einsum("ihc,hc->ih", W0.reshape(F_IN, H0, HID), as0).astype(np.float32)
    A_d0 = np.einsum("ihc,hc->ih", W0.reshape(F_IN, H0, HID), ad0).astype(np.float32)
    A_sd0 = np.concatenate([A_s0, A_d0], axis=1)  # [F_IN, 8]
    A_sd1 = np.stack([
        np.einsum("ihc,hc->ih", W1.reshape(H0 * HID, 1, HID), as1)[:, 0],
        np.einsum("ihc,hc->ih", W1.reshape(H0 * HID, 1, HID), ad1)[:, 0],
    ], axis=1).astype(np.float32)

    loop = np.arange(N, dtype=np.int32)
    src = np.concatenate([ei[0], loop])
    dst = np.concatenate([ei[1], loop])

    packs = [_pack_core(src, dst, c) for c in range(N_CORES)]
    n_sb = {h: max((len(p[h]) + GEO[h]["SBB"] - 1) // GEO[h]["SBB"] for p in packs)
            for h in ("A", "B")}

    common = {
        "W0b": W0.astype(BF16),
        "A_sd0": A_sd0.astype(BF16),
        "W1b": W1.astype(BF16),
        "A_sd1b": A_sd1.astype(BF16),
        "bias0t": np.tile(b0[None, :], (128, 1)).astype(BF16),
        "bias1t": np.tile(b1[None, :], (128, 1)).astype(BF16),
        "iota64": np.tile(np.arange(SLOTS, dtype=np.float32), (128, SUBMAX, 1))
                    .reshape(128, SUBMAX * SLOTS).astype(BF16),
        "ones_bf": np.ones((128, 1), BF16),
        "identb": np.eye(128, dtype=np.float32).astype(BF16),
    }
    in_maps = []
    for c in range(N_CORES):
        d = dict(common)
        xl = np.zeros((NLOCPAD, F_IN), np.float32)
        xl[:NPC] = x[c * NPC:(c + 1) * NPC]
        d["xT"] = np.ascontiguousarray(xl.T).astype(BF16)  # [F_IN, NLOCPAD]
        for half in ("A", "B"):
            st = _streams_for_half(packs[c][half], n_sb[half], GEO[half])
            for k, v in st.items():
                d[f"{k}_{half}"] = v.reshape(-1, v.shape[-1])
        in_maps.append(d)
    return in_maps, n_sb


# ----------------------------------------------------------------------------
# bass program
# ----------------------------------------------------------------------------

def build_program(n_sb, phases="full"):
    import concourse.bass as bass
    import concourse.bacc as bacc
    import concourse.tile as tile
    import concourse.mybir as mybir
    dt = mybir.dt

    import os as _os
    GCH = int(_os.environ.get("KGAT_GCH", "1024"))
    nc = bacc.Bacc("TRN2", target_bir_lowering=False, debug=False,
                   enable_asserts=False, num_devices=N_CORES,
                   num_swdge_queues=4,
                   dynamic_dma_scratch_size=16384 * (GCH // 1024))

    _gq = [0]

    def gather(out_ap, in_ap, idxs_ap, num_idxs, elem_size):
        # dma_gather corrupts above the SWDGE ring capacity; chunk at GCH.
        done = 0
        while done < num_idxs:
            ch = min(GCH, num_idxs - done)
            assert ch % 128 == 0
            nc.gpsimd.dma_gather(
                out_ap=out_ap[:, done // 128:(done + ch) // 128, :],
                in_ap=in_ap,
                idxs_ap=idxs_ap[:, done // 16:(done + ch) // 16],
                num_idxs=ch, num_idxs_reg=ch, elem_size=elem_size,
                queue_num=_gq[0] % 4)
            _gq[0] += 1
            done += ch

    def inp(name, shape, dtype):
        return nc.dram_tensor(name, shape, dtype, kind="ExternalInput").ap()

    xT = inp("xT", [F_IN, NLOCPAD], dt.bfloat16)
    W0b = inp("W0b", [F_IN, H0 * HID], dt.bfloat16)
    A_sd0 = inp("A_sd0", [F_IN, 2 * H0], dt.bfloat16)
    W1b = inp("W1b", [H0 * HID, HID], dt.bfloat16)
    A_sd1b = inp("A_sd1b", [H0 * HID, 2], dt.bfloat16)
    bias0t = inp("bias0t", [128, H0 * HID], dt.bfloat16)
    bias1t = inp("bias1t", [128, HID], dt.bfloat16)
    iota64 = inp("iota64", [128, SUBMAX * SLOTS], dt.bfloat16)
    ones_bf = inp("ones_bf", [128, 1], dt.bfloat16)
    identb = inp("identb", [128, 128], dt.bfloat16)
    streams = {}
    nrows = {}
    for half in ("A", "B"):
        ns, SBB, SUBh = n_sb[half], GEO[half]["SBB"], GEO[half]["SUB"]
        streams[half] = dict(
            hidx=inp(f"hidx_{half}", [ns * 128, SBE // 16], dt.int16),
            adidx=inp(f"adidx_{half}", [ns * 128, SBE // 16], dt.int16),
            slotb2=inp(f"slotb2_{half}", [ns * 128, SBB * SUBh * 2], dt.bfloat16),
            rowchunks=inp(f"rowchunks_{half}", [NCHUNK * 128, 64], dt.int16),
        )
        nrows[half] = ns * SBB * SLOTS + 128   # + zero block
    out_f = nc.dram_tensor("out", [NPC, HID], dt.float32, kind="ExternalOutput").ap()

    CH = {"A": CHA, "B": CHB}
    NR = {"A": NRA, "B": NRB}
    t0loc = {h: nc.dram_tensor(f"t0loc{h}", [CH[h], RU0], dt.bfloat16).ap()
             for h in ("A", "B")}
    tab0 = {h: nc.dram_tensor(f"tab0{h}", [NR[h], RU0], dt.bfloat16,
                              addr_space="Shared").ap() for h in ("A", "B")}
    ad0_loc = nc.dram_tensor("ad0_loc", [NLOCPAD, ADB], dt.bfloat16).ap()
    seq0 = {h: nc.dram_tensor(f"seq0{h}", [nrows[h], RU0], dt.bfloat16).ap()
            for h in ("A", "B")}
    t1loc = {h: nc.dram_tensor(f"t1loc{h}", [CH[h], RU1], dt.bfloat16).ap()
             for h in ("A", "B")}
    tab1 = {h: nc.dram_tensor(f"tab1{h}", [NR[h], RU1], dt.bfloat16,
                              addr_space="Shared").ap() for h in ("A", "B")}
    ad1_loc = nc.dram_tensor("ad1_loc", [NLOCPAD, ADB], dt.bfloat16).ap()
    seq1 = {h: nc.dram_tensor(f"seq1{h}", [nrows[h], RU1], dt.bfloat16).ap()
            for h in ("A", "B")}
    NT_A = CHA // 128   # 25 tiles in chunk A

    AF = mybir.ActivationFunctionType
    OP = mybir.AluOpType

    def pair_bcast(ap_pk2, outer, inner):
        """[128, outer, 2] -> broadcast AP [128, outer, inner, 2]."""
        return (ap_pk2.rearrange("p a b -> p a b ()")
                .rearrange("p a b u -> p a u b")
                .to_broadcast([128, outer, inner, 2]))

    with tile.TileContext(nc) as tc:
        with (
            tc.tile_pool(name="const", bufs=1) as cpool,
            tc.tile_pool(name="sbuf", bufs=3) as pool,
            tc.tile_pool(name="gath", bufs=3) as gpool,
            tc.tile_pool(name="psum", bufs=2, space="PSUM") as psum,
            tc.tile_pool(name="psB", bufs=2, space="PSUM") as psumB,
        ):
            identt = cpool.tile([128, 128], dt.bfloat16)
            nc.sync.dma_start(out=identt[:], in_=identb[:])
            W0bt = cpool.tile([128, H0 * HID], dt.bfloat16)
            nc.sync.dma_start(out=W0bt[:], in_=W0b[:])
            A_sd0t = cpool.tile([128, 2 * H0], dt.bfloat16)
            nc.sync.dma_start(out=A_sd0t[:], in_=A_sd0[:])
            W1bt = cpool.tile([128, 2, HID], dt.bfloat16)
            nc.sync.dma_start(out=W1bt[:], in_=W1b[:].rearrange("(a p) d -> p a d", p=128))
            A_sd1t = cpool.tile([128, 2, 2], dt.bfloat16)
            nc.sync.dma_start(out=A_sd1t[:], in_=A_sd1b[:].rearrange("(a p) d -> p a d", p=128))
            bias0tt = cpool.tile([128, H0 * HID], dt.bfloat16)
            nc.sync.dma_start(out=bias0tt[:], in_=bias0t[:])
            bias1tt = cpool.tile([128, HID], dt.bfloat16)
            nc.sync.dma_start(out=bias1tt[:], in_=bias1t[:])
            iota64t = cpool.tile([128, SUBMAX, SLOTS], dt.bfloat16)
            nc.sync.dma_start(out=iota64t[:],
                              in_=iota64[:].rearrange("p (a b) -> p a b", a=SUBMAX))
            ones_bft = cpool.tile([128, 1], dt.bfloat16)
            nc.sync.dma_start(out=ones_bft[:], in_=ones_bf[:])
            zeroB = cpool.tile([128, RU0], dt.bfloat16)
            nc.vector.memset(zeroB[:], 0.0)

            # zero rows at tail of each seq buffer
            for h in ("A", "B"):
                nc.sync.dma_start(out=seq0[h][nrows[h] - 128:, :], in_=zeroB[:])
                nc.sync.dma_start(out=seq1[h][nrows[h] - 128:, :], in_=zeroB[:, :RU1])

            # ---- phase A: local table0 slice + ad0 table (sharded) ----
            def phase_a_tile(t):
                r0 = t * 128
                xt = pool.tile([128, 128], dt.bfloat16, tag="xt")
                nc.sync.dma_start(out=xt[:], in_=xT[:, r0:r0 + 128])
                psH = psum.tile([128, H0 * HID], dt.float32, tag="psH")
                nc.tensor.matmul(out=psH[:], lhsT=xt[:], rhs=W0bt[:], start=True, stop=True)
                psA = psum.tile([128, 2 * H0], dt.float32, tag="psA")
                nc.tensor.matmul(out=psA[:], lhsT=xt[:], rhs=A_sd0t[:], start=True, stop=True)
                stag = pool.tile([128, RU0], dt.bfloat16, tag="stag")
                nc.vector.tensor_copy(out=stag[:, 0:H0], in_=psA[:, 0:H0])
                sv = stag[:, H0:H0 + H0 * HW0].rearrange("p (h u) -> p h u", h=H0)
                nc.vector.tensor_copy(
                    out=sv[:, :, 0:2],
                    in_=ones_bft[:].rearrange("p u -> p u ()").to_broadcast([128, H0, 2]))
                nc.vector.tensor_copy(
                    out=sv[:, :, 2:HW0],
                    in_=psH[:].rearrange("p (h u) -> p h u", h=H0))
                if t < NT_A:
                    nc.scalar.dma_start(out=t0loc["A"][r0:r0 + 128, :], in_=stag[:])
                else:
                    nc.scalar.dma_start(out=t0loc["B"][r0 - CHA:r0 - CHA + 128, :],
                                      in_=stag[:])
                adst = pool.tile([128, ADB], dt.bfloat16, tag="adst")
                nc.vector.tensor_copy(out=adst[:, 0:H0], in_=psA[:, H0:2 * H0])
                nc.scalar.dma_start(out=ad0_loc[r0:r0 + 128, :], in_=adst[:])

            def ag(src_ap, dst_ap):
                nc.gpsimd.collective_compute(
                    "AllGather", OP.bypass, replica_groups=[list(range(N_CORES))],
                    ins=[src_ap.opt()], outs=[dst_ap.opt()])

            if phases != "empty":
                for t in range(NT_A):
                    phase_a_tile(t)
                if phases in ("B", "C", "G", "full"):
                    ag(t0loc["A"], tab0["A"])
                for t in range(NT_A, NT_C):
                    phase_a_tile(t)
                if phases in ("B", "C", "G", "full"):
                    ag(t0loc["B"], tab0["B"])

            # ---- edge phase ----
            def edge_phase(layer):
                if layer == 0:
                    tabs, ad_loc, seqT, ru, nheads = tab0, ad0_loc, seq0, RU0, H0
                    as_u, hw, mo, ow = H0, HW0, H0, OW0
                else:
                    tabs, ad_loc, seqT, ru, nheads = tab1, ad1_loc, seq1, RU1, 1
                    as_u, hw, mo, ow = 1, OW1, 0, OW1
                import os as _os
                _halves = _os.environ.get("KGAT_HALVES", "AB")
                _maxsb = int(_os.environ.get("KGAT_MAXSB", "9999"))
                for half in [h for h in ("A", "B") if h in _halves]:
                    SUBh, SBB = GEO[half]["SUB"], GEO[half]["SBB"]
                    tab = tabs[half]
                    st = streams[half]
                    for s in range(min(n_sb[half], _maxsb)):
                        hix = pool.tile([128, SBE // 16], dt.int16, tag="hix")
                        nc.sync.dma_start(out=hix[:], in_=st["hidx"][s * 128:(s + 1) * 128, :])
                        aix = pool.tile([128, SBE // 16], dt.int16, tag="aix")
                        nc.sync.dma_start(out=aix[:], in_=st["adidx"][s * 128:(s + 1) * 128, :])
                        slt = pool.tile([128, SBB * SUBh, 2], dt.bfloat16, tag="slt")
                        nc.sync.dma_start(
                            out=slt[:],
                            in_=st["slotb2"][s * 128:(s + 1) * 128, :]
                                .rearrange("p (a b) -> p a b", b=2))

                        hg = gpool.tile([128, SBE // 128, ru], dt.bfloat16, tag="hg")
                        gather(hg[:], tab, hix[:], SBE, ru)
                        adg = gpool.tile([128, SBE // 128, ADB], dt.bfloat16, tag="adg")
                        gather(adg[:], ad_loc[:], aix[:], SBE, ADB)

                        # superblock-wide attention weights + one-hot build:
                        # one instruction each instead of one per block.
                        NS = SBB * SUBh
                        ev = pool.tile([128, NS, nheads], dt.bfloat16, tag="ev")
                        nc.vector.tensor_tensor(
                            out=ev[:], in0=hg[:, :, 0:nheads],
                            in1=adg[:, :, 0:nheads], op=OP.add)
                        tv = pool.tile([128, NS, nheads], dt.bfloat16, tag="tv")
                        nc.vector.tensor_scalar_mul(out=tv[:], in0=ev[:], scalar1=0.2)
                        nc.vector.tensor_tensor(out=tv[:], in0=tv[:], in1=ev[:], op=OP.max)
                        wv = pool.tile([128, NS, nheads], dt.bfloat16, tag="wv")
                        nc.scalar.activation(out=wv[:], in_=tv[:], func=AF.Exp)
                        wv2 = pool.tile([128, NS, nheads, 2], dt.bfloat16, tag="wv2")
                        nc.vector.tensor_copy(
                            out=wv2[:],
                            in_=wv[:].rearrange("p a h -> p a h ()")
                                .to_broadcast([128, NS, nheads, 2]))
                        B8 = pool.tile([128, NS, SLOTS], dt.bfloat16, tag="B8")
                        nc.vector.tensor_tensor(
                            out=B8[:].rearrange("p a (c b) -> p a c b", b=2),
                            in0=pair_bcast(slt[:], NS, SLOTS // 2),
                            in1=iota64t[:, 0, :].rearrange("p (c b) -> p c b", b=2)
                                .rearrange("p c b -> p () c b")
                                .to_broadcast([128, NS, SLOTS // 2, 2]),
                            op=OP.is_equal)

                        stage = pool.tile([128, SBB // 2, ru], dt.bfloat16, tag="stage")
                        for b in range(SBB):
                            g0 = b * SUBh
                            m = 64 * (b % 2)
                            if b % 2 == 0:
                                ps = psumB.tile([128, ow], dt.float32, tag="psB")
                            rhs = pool.tile([128, SUBh, nheads * hw], dt.bfloat16,
                                            tag="rhs")
                            for hh in range(nheads):
                                o = mo + hw * hh
                                nc.vector.tensor_tensor(
                                    out=rhs[:, :, hw * hh:hw * hh + hw]
                                        .rearrange("p a (c b) -> p a c b", b=2),
                                    in0=hg[:, g0:g0 + SUBh, o:o + hw]
                                        .rearrange("p a (c b) -> p a c b", b=2),
                                    in1=pair_bcast(wv2[:, g0:g0 + SUBh, hh, :],
                                                   SUBh, hw // 2),
                                    op=OP.mult)
                            for k in range(SUBh):
                                nc.tensor.matmul(
                                    out=ps[m:m + 64, 0:nheads * hw],
                                    lhsT=B8[:, g0 + k, :], rhs=rhs[:, k, :],
                                    start=(k == 0), stop=(k == SUBh - 1))
                            if b % 2 == 1:
                                c = b // 2
                                nc.vector.tensor_copy(out=stage[:, c, 0:ow], in_=ps[:, 0:ow])
                        r0 = s * SBB * SLOTS
                        nc.scalar.dma_start(
                            out=seqT[half][r0:r0 + SBB * SLOTS, :]
                                .rearrange("(c p) u -> p c u", p=128),
                            in_=stage[:])

            if phases in ("B", "C", "G", "full"):
                edge_phase(0)

            # ---- phase C: finalize layer-0, build table1 local slice ----
            for u in (range(NCHUNK) if phases in ("C", "G", "full") else []):
                gL = gpool.tile([128, 8, RU0], dt.bfloat16, tag="hg")
                gH = gpool.tile([128, 8, RU0], dt.bfloat16, tag="adg")
                for h, g in (("A", gL), ("B", gH)):
                    rix = pool.tile([128, 64], dt.int16, tag="rix")
                    nc.sync.dma_start(out=rix[:],
                                      in_=streams[h]["rowchunks"][u * 128:(u + 1) * 128, :])
                    gather(g[:], seq0[h][:], rix[:], 1024, RU0)
                for tt in range(8):
                    t = u * 8 + tt
                    if t >= NT_C:
                        break
                    r0 = t * 128
                    cnt = 128 if t < NT_C - 1 else LAST_C
                    o = pool.tile([128, OW0], dt.bfloat16, tag="oC")
                    nc.vector.tensor_tensor(out=o[:], in0=gL[:, tt, 0:OW0],
                                            in1=gH[:, tt, 0:OW0], op=OP.add)
                    ov = o[:].rearrange("p (h u) -> p h u", h=H0)
                    rec = pool.tile([128, H0], dt.float32, tag="rec")
                    nc.vector.reciprocal(out=rec[:],
                                         in_=ov[:, :, 0:1].rearrange("p h u -> p (h u)"))
                    rec2 = pool.tile([128, H0, 2], dt.bfloat16, tag="rec2")
                    nc.vector.tensor_copy(
                        out=rec2[:],
                        in_=rec[:].rearrange("p h -> p h ()").to_broadcast([128, H0, 2]))
                    z = pool.tile([128, H0, HID], dt.bfloat16, tag="z")
                    nc.vector.tensor_tensor(
                        out=z[:].rearrange("p h (c b) -> p h c b", b=2),
                        in0=ov[:, :, 2:HW0].rearrange("p h (c b) -> p h c b", b=2),
                        in1=pair_bcast(rec2[:], H0, HID // 2),
                        op=OP.mult)
                    zf = z[:].rearrange("p h u -> p (h u)")
                    nc.vector.tensor_tensor(out=zf, in0=zf, in1=bias0tt[:], op=OP.add)
                    zm = pool.tile([128, H0 * HID], dt.bfloat16, tag="zm")
                    nc.vector.tensor_scalar_min(out=zm[:], in0=zf, scalar1=0.0)
                    qe = pool.tile([128, H0 * HID], dt.bfloat16, tag="qe")
                    nc.scalar.activation(out=qe[:], in_=zm[:], func=AF.Exp)
                    nc.vector.tensor_scalar(out=zf, in0=zf, scalar1=0.0, scalar2=-1.0,
                                            op0=OP.max, op1=OP.add)
                    nc.vector.tensor_tensor(out=zf, in0=zf, in1=qe[:], op=OP.add)
                    psH1 = psum.tile([128, HID], dt.float32, tag="psH")
                    psA1 = psum.tile([128, 2], dt.float32, tag="psA")
                    for ch in range(2):
                        psT = psum.tile([128, 128], dt.bfloat16, tag="psT")
                        nc.tensor.transpose(out=psT[:],
                                            in_=zf[:, 128 * ch:128 * ch + 128],
                                            identity=identt[:])
                        zTb = pool.tile([128, 128], dt.bfloat16, tag="zTb")
                        nc.vector.tensor_copy(out=zTb[:], in_=psT[:])
                        nc.tensor.matmul(out=psH1[:], lhsT=zTb[:], rhs=W1bt[:, ch, :],
                                         start=(ch == 0), stop=(ch == 1))
                        nc.tensor.matmul(out=psA1[:], lhsT=zTb[:], rhs=A_sd1t[:, ch, :],
                                         start=(ch == 0), stop=(ch == 1))
                    t1s = pool.tile([128, RU1], dt.bfloat16, tag="t1s")
                    nc.vector.tensor_copy(
                        out=t1s[:, 0:2],
                        in_=psA1[:, 0:1].to_broadcast([128, 2]))
                    nc.vector.tensor_copy(
                        out=t1s[:, 2:4],
                        in_=ones_bft[:].to_broadcast([128, 2]))
                    nc.vector.tensor_copy(out=t1s[:, 4:4 + HID], in_=psH1[:])
                    if t < NT_A:
                        nc.scalar.dma_start(out=t1loc["A"][r0:r0 + cnt, :],
                                          in_=t1s[0:cnt, :])
                    else:
                        nc.scalar.dma_start(out=t1loc["B"][r0 - CHA:r0 - CHA + cnt, :],
                                          in_=t1s[0:cnt, :])
                    a1s = pool.tile([128, ADB], dt.bfloat16, tag="adst")
                    nc.vector.tensor_copy(out=a1s[:, 0:1], in_=psA1[:, 1:2])
                    nc.scalar.dma_start(out=ad1_loc[r0:r0 + 128, :], in_=a1s[:])
                    if t == NT_A - 1 and phases in ("G", "full"):
                        ag(t1loc["A"], tab1["A"])   # overlap with B-chunk finalize

            # ---- AllGather table1 (B chunk) ----
            if phases in ("G", "full"):
                ag(t1loc["B"], tab1["B"])

            if phases == "full":
                edge_phase(1)

            # ---- phase E: finalize layer-1 ----
            for u in (range(NCHUNK) if phases == "full" else []):
                gL = gpool.tile([128, 8, RU1], dt.bfloat16, tag="hg")
                gH = gpool.tile([128, 8, RU1], dt.bfloat16, tag="adg")
                for h, g in (("A", gL), ("B", gH)):
                    rix = pool.tile([128, 64], dt.int16, tag="rix")
                    nc.sync.dma_start(out=rix[:],
                                      in_=streams[h]["rowchunks"][u * 128:(u + 1) * 128, :])
                    gather(g[:], seq1[h][:], rix[:], 1024, RU1)
                for tt in range(8):
                    t = u * 8 + tt
                    if t >= NT_C:
                        break
                    r0 = t * 128
                    cnt = 128 if t < NT_C - 1 else LAST_C
                    o = pool.tile([128, OW1], dt.bfloat16, tag="o1")
                    nc.vector.tensor_tensor(out=o[:], in0=gL[:, tt, 0:OW1],
                                            in1=gH[:, tt, 0:OW1], op=OP.add)
                    rec = pool.tile([128, 1], dt.float32, tag="rec1")
                    nc.vector.reciprocal(out=rec[:], in_=o[:, 2:3])
                    rec2 = pool.tile([128, 2], dt.bfloat16, tag="rec12")
                    nc.vector.tensor_copy(out=rec2[:], in_=rec[:].to_broadcast([128, 2]))
                    res = pool.tile([128, HID], dt.bfloat16, tag="res")
                    nc.vector.tensor_tensor(
                        out=res[:].rearrange("p (c b) -> p c b", b=2),
                        in0=o[:, 4:4 + HID].rearrange("p (c b) -> p c b", b=2),
                        in1=rec2[:].rearrange("p b -> p b ()").rearrange("p b u -> p u b")
                            .to_broadcast([128, HID // 2, 2]),
                        op=OP.mult)
                    resf = pool.tile([128, HID], dt.float32, tag="resf")
                    nc.vector.tensor_tensor(out=resf[:], in0=res[:], in1=bias1tt[:], op=OP.add)
                    nc.scalar.dma_start(out=out_f[r0:r0 + cnt, :], in_=resf[0:cnt, :])

            if phases != "full":
                for t in range(NT_C):
                    r0 = t * 128
                    cnt = 128 if t < NT_C - 1 else LAST_C
                    zf32 = cpool.tile([128, HID], dt.float32)
                    nc.vector.memset(zf32[:], 0.0)
                    nc.sync.dma_start(out=out_f[r0:r0 + cnt, :], in_=zf32[0:cnt, :])

    nc.compile()
    return nc


def kernel(**inputs):
    import os
    from concourse import bass_utils
    in_maps, n_sb = host_prepare(inputs)
    phases = os.environ.get("KGAT_PHASES", "full")
    key = (n_sb["A"], n_sb["B"], phases)
    if key not in _prog_cache:
        _prog_cache[key] = build_program(n_sb, phases)
    nc = _prog_cache[key]
    res = bass_utils.run_bass_kernel_spmd(nc, in_maps, core_ids=list(range(N_CORES)))
    out = np.concatenate([np.asarray(res.results[c]["out"]) for c in range(N_CORES)], axis=0)
    return out.astype(np.float32)


# revision 35
# speedup vs baseline: 1.0580x; 1.0048x over previous
"""Trainium2 Bass kernel for 2-layer GAT (nn_GAT_47957604827269).

Strategy: partition nodes across 8 cores by dst range. Per layer:
  - per-core table slice build (local x^T tiles -> PE matmuls), AllGather to
    a full per-node feature table in DRAM: row = [as | (1,1,h)*H] in bf16,
    256B-aligned rows for the dma_gather ucode op.
  - per-edge gather (dma_gather, int16 idx -> edges split by table row <
    32768), attention weights w = exp(leaky_relu(as[src] + ad[dst])) on-chip
    (ad gathered from a core-local bf16 table), aggregation via one-hot
    slot-matmul on the PE: B.T @ (w * [1|1|h]) giving per-node numerators and
    (via the duplicated ones columns) denominators in one pass.
  - block results land contiguously in DRAM ("seq" buffers, bf16); the
    finalize pass gathers each node's L/H partial rows, adds, normalizes.

DVE ops use 16-bit dtypes with pair-replicated scalars so the per-element
broadcast multiplies hit the DVE 2x packed mode.
"""
import numpy as np
import ml_dtypes

BF16 = ml_dtypes.bfloat16

# ---- problem constants (hardcoded per contract) ----
N = 50000
F_IN = 128
HID = 64
H0 = 4
N_CORES = 8
NPC = N // N_CORES            # 6250
NLOCPAD = 6272                # 49*128: padded rows per core (table row space)
CHA = 3200                    # chunk-A rows per core (25 tiles)
CHB = 3072                    # chunk-B rows per core (24 tiles)
NRA = CHA * N_CORES           # 25600 rows in table chunk A (< 2^15 for int16)
NRB = CHB * N_CORES           # 24576 rows in table chunk B
SLOTS = 64
TRASH = SLOTS - 1             # 63
GEO = {"A": dict(BLK=512, SUB=4, SBB=8),    # 4096 edges / superblock
       "B": dict(BLK=512, SUB=4, SBB=8)}
SBE = 4096
SUBMAX = 6
RU0 = 384                     # bf16 units per table0 row (768B); 268 used
RU1 = 128                     # table1 row units (256B); 68 used
ADB = 128                     # ad table row bf16 units (256B)
OW0 = 264                     # seq0 used cols (bf16), row stride RU0
OW1 = 68                      # seq1 used cols (bf16), row stride RU1
HW0 = 66                      # layer-0 per-head block: [1,1,h*64]
NT_C = NLOCPAD // 128         # 49
LAST_C = NPC - 48 * 128       # 106
NCHUNK = 7                    # finalize gather chunks of 1024 nodes

_prog_cache = {}


def _wrap16(idx, pad_to=None):
    """ucode idx layout: idx i at [i%16, i//16], replicated to 128 partitions."""
    idx = np.asarray(idx, np.int16)
    if pad_to is not None and len(idx) < pad_to:
        idx = np.concatenate([idx, np.zeros(pad_to - len(idx), np.int16)])
    n = len(idx)
    a = idx.reshape(n // 16, 16).T.copy()
    return np.tile(a, (8, 1))


def _pack_half(ss, dd, geo):
    """Greedy-pack edges (dst-sorted local) into BLK-edge / 63-slot blocks."""
    BLK = geo["BLK"]
    blocks = []
    if len(ss):
        uniq, starts = np.unique(dd, return_index=True)
        ends = np.append(starts[1:], len(dd))
        cur_s, cur_nodes, cur_slot = [], [], []
        for nd, st, en in zip(uniq, starts, ends):
            deg = en - st
            if len(cur_s) + deg > BLK or len(cur_nodes) >= TRASH:
                blocks.append((cur_s, cur_slot, cur_nodes))
                cur_s, cur_nodes, cur_slot = [], [], []
            sl = len(cur_nodes)
            cur_nodes.append(nd)
            cur_s.extend(ss[st:en])
            cur_slot.extend([sl] * deg)
        if cur_s:
            blocks.append((cur_s, cur_slot, cur_nodes))
    return blocks


def _pack_core(src, dst, core):
    lo, hi = core * NPC, (core + 1) * NPC
    m = (dst >= lo) & (dst < hi)
    s = src[m]
    d_loc = dst[m] - lo
    order = np.argsort(d_loc, kind="stable")
    s, d_loc = s[order], d_loc[order]
    sc, so = s // NPC, s % NPC          # owning core, local offset
    in_a = so < CHA
    out = {}
    for half, sel in (("A", in_a), ("B", ~in_a)):
        ss = np.where(in_a, sc * CHA + so, sc * CHB + (so - CHA))[sel]
        out[half] = _pack_half(ss, d_loc[sel], GEO[half])
    return out


def _streams_for_half(blocks, n_sb_target, geo):
    BLK, SBB = geo["BLK"], geo["SBB"]
    nbt = n_sb_target * SBB
    src_b = np.zeros((nbt, BLK), np.int32)
    slot_b = np.full((nbt, BLK), TRASH, np.int32)
    dloc_b = np.zeros((nbt, BLK), np.int32)
    rowpos = np.full(NLOCPAD + 1024, nbt * SLOTS, np.int32)  # default: zero row
    for j, (s_, sl_, nds) in enumerate(blocks):
        k = len(s_)
        src_b[j, :k] = s_
        slot_b[j, :k] = sl_
        nda = np.asarray(nds, np.int32)
        dloc_b[j, :k] = nda[np.asarray(sl_, np.int32)]
        rowpos[nda] = j * SLOTS + np.arange(len(nds), dtype=np.int32)
    src_sb = src_b.reshape(n_sb_target, SBB * BLK)
    dloc_sb = dloc_b.reshape(n_sb_target, SBB * BLK)
    hidx = np.stack([_wrap16(r.astype(np.int16)) for r in src_sb])
    adidx = np.stack([_wrap16(r.astype(np.int16)) for r in dloc_sb])
    # paired bf16 slot stream: [n_sb, 128, SBB*SUB, 2]
    slotb = (slot_b.reshape(n_sb_target, SBB * (BLK // 128), 128)
             .transpose(0, 2, 1))                 # [n_sb, 128, SBB*SUB]
    slotb2 = np.repeat(slotb.astype(BF16), 2, axis=2).reshape(n_sb_target, 128, -1)
    # finalize gather idx: chunks of 1024 node ids
    rows = rowpos[:NCHUNK * 1024].astype(np.int16)
    rowchunks = np.stack([_wrap16(rows[u * 1024:(u + 1) * 1024])
                          for u in range(NCHUNK)])
    return dict(hidx=hidx.astype(np.int16), adidx=adidx.astype(np.int16),
                slotb2=slotb2, rowchunks=rowchunks.astype(np.int16))


def host_prepare(inputs):
    x = np.ascontiguousarray(np.asarray(inputs["x"], np.float32))
    ei = np.asarray(inputs["edge_index"], np.int32)
    W0 = np.asarray(inputs["W0"], np.float32)
    as0 = np.asarray(inputs["att_src0"], np.float32)
    ad0 = np.asarray(inputs["att_dst0"], np.float32)
    b0 = np.asarray(inputs["bias0"], np.float32)
    W1 = np.asarray(inputs["W1"], np.float32)
    as1 = np.asarray(inputs["att_src1"], np.float32)
    ad1 = np.asarray(inputs["att_dst1"], np.float32)
    b1 = np.asarray(inputs["bias1"], np.float32)

    A_s0 = np.einsum("ihc,hc->ih", W0.reshape(F_IN, H0, HID), as0).astype(np.float32)
    A_d0 = np.einsum("ihc,hc->ih", W0.reshape(F_IN, H0, HID), ad0).astype(np.float32)
    A_sd0 = np.concatenate([A_s0, A_d0], axis=1)  # [F_IN, 8]
    A_sd1 = np.stack([
        np.einsum("ihc,hc->ih", W1.reshape(H0 * HID, 1, HID), as1)[:, 0],
        np.einsum("ihc,hc->ih", W1.reshape(H0 * HID, 1, HID), ad1)[:, 0],
    ], axis=1).astype(np.float32)

    loop = np.arange(N, dtype=np.int32)
    src = np.concatenate([ei[0], loop])
    dst = np.concatenate([ei[1], loop])

    packs = [_pack_core(src, dst, c) for c in range(N_CORES)]
    n_sb = {h: max((len(p[h]) + GEO[h]["SBB"] - 1) // GEO[h]["SBB"] for p in packs)
            for h in ("A", "B")}

    common = {
        "W0b": W0.astype(BF16),
        "A_sd0": A_sd0.astype(BF16),
        "W1b": W1.astype(BF16),
        "A_sd1b": A_sd1.astype(BF16),
        "bias0t": np.tile(b0[None, :], (128, 1)).astype(BF16),
        "bias1t": np.tile(b1[None, :], (128, 1)).astype(BF16),
        "iota64": np.tile(np.arange(SLOTS, dtype=np.float32), (128, SUBMAX, 1))
                    .reshape(128, SUBMAX * SLOTS).astype(BF16),
        "ones_bf": np.ones((128, 1), BF16),
        "identb": np.eye(128, dtype=np.float32).astype(BF16),
    }
    in_maps = []
    for c in range(N_CORES):
        d = dict(common)
        xl = np.zeros((NLOCPAD, F_IN), np.float32)
        xl[:NPC] = x[c * NPC:(c + 1) * NPC]
        d["xT"] = np.ascontiguousarray(xl.T).astype(BF16)  # [F_IN, NLOCPAD]
        for half in ("A", "B"):
            st = _streams_for_half(packs[c][half], n_sb[half], GEO[half])
            for k, v in st.items():
                d[f"{k}_{half}"] = v.reshape(-1, v.shape[-1])
        in_maps.append(d)
    return in_maps, n_sb


# ----------------------------------------------------------------------------
# bass program
# ----------------------------------------------------------------------------

def build_program(n_sb, phases="full"):
    import concourse.bass as bass
    import concourse.bacc as bacc
    import concourse.tile as tile
    import concourse.mybir as mybir
    dt = mybir.dt

    import os as _os
    GCH = int(_os.environ.get("KGAT_GCH", "1024"))
    nc = bacc.Bacc("TRN2", target_bir_lowering=False, debug=False,
                   enable_asserts=False, num_devices=N_CORES,
                   num_swdge_queues=4,
                   dynamic_dma_scratch_size=16384 * (GCH // 1024))

    _gq = [0]

    def gather(out_ap, in_ap, idxs_ap, num_idxs, elem_size):
        # dma_gather corrupts above the SWDGE ring capacity; chunk at GCH.
        done = 0
        while done < num_idxs:
            ch = min(GCH, num_idxs - done)
            assert ch % 128 == 0
            nc.gpsimd.dma_gather(
                out_ap=out_ap[:, done // 128:(done + ch) // 128, :],
                in_ap=in_ap,
                idxs_ap=idxs_ap[:, done // 16:(done + ch) // 16],
                num_idxs=ch, num_idxs_reg=ch, elem_size=elem_size,
                queue_num=_gq[0] % 4)
            _gq[0] += 1
            done += ch

    def inp(name, shape, dtype):
        return nc.dram_tensor(name, shape, dtype, kind="ExternalInput").ap()

    xT = inp("xT", [F_IN, NLOCPAD], dt.bfloat16)
    W0b = inp("W0b", [F_IN, H0 * HID], dt.bfloat16)
    A_sd0 = inp("A_sd0", [F_IN, 2 * H0], dt.bfloat16)
    W1b = inp("W1b", [H0 * HID, HID], dt.bfloat16)
    A_sd1b = inp("A_sd1b", [H0 * HID, 2], dt.bfloat16)
    bias0t = inp("bias0t", [128, H0 * HID], dt.bfloat16)
    bias1t = inp("bias1t", [128, HID], dt.bfloat16)
    iota64 = inp("iota64", [128, SUBMAX * SLOTS], dt.bfloat16)
    ones_bf = inp("ones_bf", [128, 1], dt.bfloat16)
    identb = inp("identb", [128, 128], dt.bfloat16)
    streams = {}
    nrows = {}
    for half in ("A", "B"):
        ns, SBB, SUBh = n_sb[half], GEO[half]["SBB"], GEO[half]["SUB"]
        streams[half] = dict(
            hidx=inp(f"hidx_{half}", [ns * 128, SBE // 16], dt.int16),
            adidx=inp(f"adidx_{half}", [ns * 128, SBE // 16], dt.int16),
            slotb2=inp(f"slotb2_{half}", [ns * 128, SBB * SUBh * 2], dt.bfloat16),
            rowchunks=inp(f"rowchunks_{half}", [NCHUNK * 128, 64], dt.int16),
        )
        nrows[half] = ns * SBB * SLOTS + 128   # + zero block
    out_f = nc.dram_tensor("out", [NPC, HID], dt.float32, kind="ExternalOutput").ap()

    CH = {"A": CHA, "B": CHB}
    NR = {"A": NRA, "B": NRB}
    t0loc = {h: nc.dram_tensor(f"t0loc{h}", [CH[h], RU0], dt.bfloat16).ap()
             for h in ("A", "B")}
    tab0 = {h: nc.dram_tensor(f"tab0{h}", [NR[h], RU0], dt.bfloat16,
                              addr_space="Shared").ap() for h in ("A", "B")}
    ad0_loc = nc.dram_tensor("ad0_loc", [NLOCPAD, ADB], dt.bfloat16).ap()
    seq0 = {h: nc.dram_tensor(f"seq0{h}", [nrows[h], RU0], dt.bfloat16).ap()
            for h in ("A", "B")}
    t1loc = {h: nc.dram_tensor(f"t1loc{h}", [CH[h], RU1], dt.bfloat16).ap()
             for h in ("A", "B")}
    tab1 = {h: nc.dram_tensor(f"tab1{h}", [NR[h], RU1], dt.bfloat16,
                              addr_space="Shared").ap() for h in ("A", "B")}
    ad1_loc = nc.dram_tensor("ad1_loc", [NLOCPAD, ADB], dt.bfloat16).ap()
    seq1 = {h: nc.dram_tensor(f"seq1{h}", [nrows[h], RU1], dt.bfloat16).ap()
            for h in ("A", "B")}
    NT_A = CHA // 128   # 25 tiles in chunk A

    AF = mybir.ActivationFunctionType
    OP = mybir.AluOpType

    def pair_bcast(ap_pk2, outer, inner):
        """[128, outer, 2] -> broadcast AP [128, outer, inner, 2]."""
        return (ap_pk2.rearrange("p a b -> p a b ()")
                .rearrange("p a b u -> p a u b")
                .to_broadcast([128, outer, inner, 2]))

    with tile.TileContext(nc) as tc:
        with (
            tc.tile_pool(name="const", bufs=1) as cpool,
            tc.tile_pool(name="sbuf", bufs=3) as pool,
            tc.tile_pool(name="gath", bufs=3) as gpool,
            tc.tile_pool(name="psum", bufs=2, space="PSUM") as psum,
            tc.tile_pool(name="psB", bufs=2, space="PSUM") as psumB,
        ):
            identt = cpool.tile([128, 128], dt.bfloat16)
            nc.sync.dma_start(out=identt[:], in_=identb[:])
            W0bt = cpool.tile([128, H0 * HID], dt.bfloat16)
            nc.sync.dma_start(out=W0bt[:], in_=W0b[:])
            A_sd0t = cpool.tile([128, 2 * H0], dt.bfloat16)
            nc.sync.dma_start(out=A_sd0t[:], in_=A_sd0[:])
            W1bt = cpool.tile([128, 2, HID], dt.bfloat16)
            nc.sync.dma_start(out=W1bt[:], in_=W1b[:].rearrange("(a p) d -> p a d", p=128))
            A_sd1t = cpool.tile([128, 2, 2], dt.bfloat16)
            nc.sync.dma_start(out=A_sd1t[:], in_=A_sd1b[:].rearrange("(a p) d -> p a d", p=128))
            bias0tt = cpool.tile([128, H0 * HID], dt.bfloat16)
            nc.sync.dma_start(out=bias0tt[:], in_=bias0t[:])
            bias1tt = cpool.tile([128, HID], dt.bfloat16)
            nc.sync.dma_start(out=bias1tt[:], in_=bias1t[:])
            iota64t = cpool.tile([128, SUBMAX, SLOTS], dt.bfloat16)
            nc.sync.dma_start(out=iota64t[:],
                              in_=iota64[:].rearrange("p (a b) -> p a b", a=SUBMAX))
            ones_bft = cpool.tile([128, 1], dt.bfloat16)
            nc.sync.dma_start(out=ones_bft[:], in_=ones_bf[:])
            zeroB = cpool.tile([128, RU0], dt.bfloat16)
            nc.vector.memset(zeroB[:], 0.0)

            # zero rows at tail of each seq buffer
            for h in ("A", "B"):
                nc.sync.dma_start(out=seq0[h][nrows[h] - 128:, :], in_=zeroB[:])
                nc.sync.dma_start(out=seq1[h][nrows[h] - 128:, :], in_=zeroB[:, :RU1])

            # ---- phase A: local table0 slice + ad0 table (sharded) ----
            def phase_a_tile(t):
                r0 = t * 128
                xt = pool.tile([128, 128], dt.bfloat16, tag="xt")
                nc.sync.dma_start(out=xt[:], in_=xT[:, r0:r0 + 128])
                psH = psum.tile([128, H0 * HID], dt.float32, tag="psH")
                nc.tensor.matmul(out=psH[:], lhsT=xt[:], rhs=W0bt[:], start=True, stop=True)
                psA = psum.tile([128, 2 * H0], dt.float32, tag="psA")
                nc.tensor.matmul(out=psA[:], lhsT=xt[:], rhs=A_sd0t[:], start=True, stop=True)
                stag = pool.tile([128, RU0], dt.bfloat16, tag="stag")
                nc.vector.tensor_copy(out=stag[:, 0:H0], in_=psA[:, 0:H0])
                sv = stag[:, H0:H0 + H0 * HW0].rearrange("p (h u) -> p h u", h=H0)
                nc.vector.tensor_copy(
                    out=sv[:, :, 0:2],
                    in_=ones_bft[:].rearrange("p u -> p u ()").to_broadcast([128, H0, 2]))
                nc.vector.tensor_copy(
                    out=sv[:, :, 2:HW0],
                    in_=psH[:].rearrange("p (h u) -> p h u", h=H0))
                if t < NT_A:
                    nc.scalar.dma_start(out=t0loc["A"][r0:r0 + 128, :], in_=stag[:])
                else:
                    nc.scalar.dma_start(out=t0loc["B"][r0 - CHA:r0 - CHA + 128, :],
                                      in_=stag[:])
                adst = pool.tile([128, ADB], dt.bfloat16, tag="adst")
                nc.vector.tensor_copy(out=adst[:, 0:H0], in_=psA[:, H0:2 * H0])
                nc.scalar.dma_start(out=ad0_loc[r0:r0 + 128, :], in_=adst[:])

            def ag(src_ap, dst_ap):
                nc.gpsimd.collective_compute(
                    "AllGather", OP.bypass, replica_groups=[list(range(N_CORES))],
                    ins=[src_ap.opt()], outs=[dst_ap.opt()])

            if phases != "empty":
                for t in range(NT_A):
                    phase_a_tile(t)
                if phases in ("B", "C", "G", "full"):
                    ag(t0loc["A"], tab0["A"])
                for t in range(NT_A, NT_C):
                    phase_a_tile(t)
                if phases in ("B", "C", "G", "full"):
                    ag(t0loc["B"], tab0["B"])

            # ---- edge phase ----
            def edge_phase(layer):
                if layer == 0:
                    tabs, ad_loc, seqT, ru, nheads = tab0, ad0_loc, seq0, RU0, H0
                    as_u, hw, mo, ow = H0, HW0, H0, OW0
                else:
                    tabs, ad_loc, seqT, ru, nheads = tab1, ad1_loc, seq1, RU1, 1
                    as_u, hw, mo, ow = 1, OW1, 0, OW1
                import os as _os
                _halves = _os.environ.get("KGAT_HALVES", "AB")
                _maxsb = int(_os.environ.get("KGAT_MAXSB", "9999"))
                for half in [h for h in ("A", "B") if h in _halves]:
                    SUBh, SBB = GEO[half]["SUB"], GEO[half]["SBB"]
                    tab = tabs[half]
                    st = streams[half]
                    for s in range(min(n_sb[half], _maxsb)):
                        hix = pool.tile([128, SBE // 16], dt.int16, tag="hix")
                        nc.sync.dma_start(out=hix[:], in_=st["hidx"][s * 128:(s + 1) * 128, :])
                        aix = pool.tile([128, SBE // 16], dt.int16, tag="aix")
                        nc.sync.dma_start(out=aix[:], in_=st["adidx"][s * 128:(s + 1) * 128, :])
                        slt = pool.tile([128, SBB * SUBh, 2], dt.bfloat16, tag="slt")
                        nc.sync.dma_start(
                            out=slt[:],
                            in_=st["slotb2"][s * 128:(s + 1) * 128, :]
                                .rearrange("p (a b) -> p a b", b=2))

                        hg = gpool.tile([128, SBE // 128, ru], dt.bfloat16, tag="hg")
                        gather(hg[:], tab, hix[:], SBE, ru)
                        adg = gpool.tile([128, SBE // 128, ADB], dt.bfloat16, tag="adg")
                        gather(adg[:], ad_loc[:], aix[:], SBE, ADB)

                        # superblock-wide attention weights + one-hot build:
                        # one instruction each instead of one per block.
                        NS = SBB * SUBh
                        ev = pool.tile([128, NS, nheads], dt.bfloat16, tag="ev")
                        nc.vector.tensor_tensor(
                            out=ev[:], in0=hg[:, :, 0:nheads],
                            in1=adg[:, :, 0:nheads], op=OP.add)
                        tv = pool.tile([128, NS, nheads], dt.bfloat16, tag="tv")
                        nc.vector.tensor_scalar_mul(out=tv[:], in0=ev[:], scalar1=0.2)
                        nc.vector.tensor_tensor(out=tv[:], in0=tv[:], in1=ev[:], op=OP.max)
                        wv = pool.tile([128, NS, nheads], dt.bfloat16, tag="wv")
                        nc.scalar.activation(out=wv[:], in_=tv[:], func=AF.Exp)
                        wv2 = pool.tile([128, NS, nheads, 2], dt.bfloat16, tag="wv2")
                        nc.vector.tensor_copy(
                            out=wv2[:],
                            in_=wv[:].rearrange("p a h -> p a h ()")
                                .to_broadcast([128, NS, nheads, 2]))
                        B8 = pool.tile([128, NS, SLOTS], dt.bfloat16, tag="B8")
                        nc.vector.tensor_tensor(
                            out=B8[:].rearrange("p a (c b) -> p a c b", b=2),
                            in0=pair_bcast(slt[:], NS, SLOTS // 2),
                            in1=iota64t[:, 0, :].rearrange("p (c b) -> p c b", b=2)
                                .rearrange("p c b -> p () c b")
                                .to_broadcast([128, NS, SLOTS // 2, 2]),
                            op=OP.is_equal)

                        # superblock-wide weighted rhs: one multiply per head
                        rhs = pool.tile([128, NS, nheads * hw], dt.bfloat16, tag="rhs")
                        for hh in range(nheads):
                            o = mo + hw * hh
                            nc.vector.tensor_tensor(
                                out=rhs[:, :, hw * hh:hw * hh + hw]
                                    .rearrange("p a (c b) -> p a c b", b=2),
                                in0=hg[:, :, o:o + hw]
                                    .rearrange("p a (c b) -> p a c b", b=2),
                                in1=pair_bcast(wv2[:, :, hh, :], NS, hw // 2),
                                op=OP.mult)

                        stage = pool.tile([128, SBB // 2, ru], dt.bfloat16, tag="stage")
                        for b in range(SBB):
                            g0 = b * SUBh
                            m = 64 * (b % 2)
                            if b % 2 == 0:
                                ps = psumB.tile([128, ow], dt.float32, tag="psB")
                            for k in range(SUBh):
                                nc.tensor.matmul(
                                    out=ps[m:m + 64, 0:nheads * hw],
                                    lhsT=B8[:, g0 + k, :], rhs=rhs[:, g0 + k, :],
                                    start=(k == 0), stop=(k == SUBh - 1))
                            if b % 2 == 1:
                                c = b // 2
                                nc.vector.tensor_copy(out=stage[:, c, 0:ow], in_=ps[:, 0:ow])
                        r0 = s * SBB * SLOTS
                        nc.scalar.dma_start(
                            out=seqT[half][r0:r0 + SBB * SLOTS, :]
                                .rearrange("(c p) u -> p c u", p=128),
                            in_=stage[:])

            if phases in ("B", "C", "G", "full"):
                edge_phase(0)

            # ---- phase C: finalize layer-0, build table1 local slice ----
            for u in (range(NCHUNK) if phases in ("C", "G", "full") else []):
                gL = gpool.tile([128, 8, RU0], dt.bfloat16, tag="hg")
                gH = gpool.tile([128, 8, RU0], dt.bfloat16, tag="adg")
                for h, g in (("A", gL), ("B", gH)):
                    rix = pool.tile([128, 64], dt.int16, tag="rix")
                    nc.sync.dma_start(out=rix[:],
                                      in_=streams[h]["rowchunks"][u * 128:(u + 1) * 128, :])
                    gather(g[:], seq0[h][:], rix[:], 1024, RU0)
                for tt in range(8):
                    t = u * 8 + tt
                    if t >= NT_C:
                        break
                    r0 = t * 128
                    cnt = 128 if t < NT_C - 1 else LAST_C
                    o = pool.tile([128, OW0], dt.bfloat16, tag="oC")
                    nc.vector.tensor_tensor(out=o[:], in0=gL[:, tt, 0:OW0],
                                            in1=gH[:, tt, 0:OW0], op=OP.add)
                    ov = o[:].rearrange("p (h u) -> p h u", h=H0)
                    rec = pool.tile([128, H0], dt.float32, tag="rec")
                    nc.vector.reciprocal(out=rec[:],
                                         in_=ov[:, :, 0:1].rearrange("p h u -> p (h u)"))
                    rec2 = pool.tile([128, H0, 2], dt.bfloat16, tag="rec2")
                    nc.vector.tensor_copy(
                        out=rec2[:],
                        in_=rec[:].rearrange("p h -> p h ()").to_broadcast([128, H0, 2]))
                    z = pool.tile([128, H0, HID], dt.bfloat16, tag="z")
                    nc.vector.tensor_tensor(
                        out=z[:].rearrange("p h (c b) -> p h c b", b=2),
                        in0=ov[:, :, 2:HW0].rearrange("p h (c b) -> p h c b", b=2),
                        in1=pair_bcast(rec2[:], H0, HID // 2),
                        op=OP.mult)
                    zf = z[:].rearrange("p h u -> p (h u)")
                    nc.vector.tensor_tensor(out=zf, in0=zf, in1=bias0tt[:], op=OP.add)
                    zm = pool.tile([128, H0 * HID], dt.bfloat16, tag="zm")
                    nc.vector.tensor_scalar_min(out=zm[:], in0=zf, scalar1=0.0)
                    qe = pool.tile([128, H0 * HID], dt.bfloat16, tag="qe")
                    nc.scalar.activation(out=qe[:], in_=zm[:], func=AF.Exp)
                    nc.vector.tensor_scalar(out=zf, in0=zf, scalar1=0.0, scalar2=-1.0,
                                            op0=OP.max, op1=OP.add)
                    nc.vector.tensor_tensor(out=zf, in0=zf, in1=qe[:], op=OP.add)
                    psH1 = psum.tile([128, HID], dt.float32, tag="psH")
                    psA1 = psum.tile([128, 2], dt.float32, tag="psA")
                    for ch in range(2):
                        psT = psum.tile([128, 128], dt.bfloat16, tag="psT")
                        nc.tensor.transpose(out=psT[:],
                                            in_=zf[:, 128 * ch:128 * ch + 128],
                                            identity=identt[:])
                        zTb = pool.tile([128, 128], dt.bfloat16, tag="zTb")
                        nc.vector.tensor_copy(out=zTb[:], in_=psT[:])
                        nc.tensor.matmul(out=psH1[:], lhsT=zTb[:], rhs=W1bt[:, ch, :],
                                         start=(ch == 0), stop=(ch == 1))
                        nc.tensor.matmul(out=psA1[:], lhsT=zTb[:], rhs=A_sd1t[:, ch, :],
                                         start=(ch == 0), stop=(ch == 1))
                    t1s = pool.tile([128, RU1], dt.bfloat16, tag="t1s")
                    nc.vector.tensor_copy(
                        out=t1s[:, 0:2],
                        in_=psA1[:, 0:1].to_broadcast([128, 2]))
                    nc.vector.tensor_copy(
                        out=t1s[:, 2:4],
                        in_=ones_bft[:].to_broadcast([128, 2]))
                    nc.vector.tensor_copy(out=t1s[:, 4:4 + HID], in_=psH1[:])
                    if t < NT_A:
                        nc.scalar.dma_start(out=t1loc["A"][r0:r0 + cnt, :],
                                          in_=t1s[0:cnt, :])
                    else:
                        nc.scalar.dma_start(out=t1loc["B"][r0 - CHA:r0 - CHA + cnt, :],
                                          in_=t1s[0:cnt, :])
                    a1s = pool.tile([128, ADB], dt.bfloat16, tag="adst")
                    nc.vector.tensor_copy(out=a1s[:, 0:1], in_=psA1[:, 1:2])
                    nc.scalar.dma_start(out=ad1_loc[r0:r0 + 128, :], in_=a1s[:])
                    if t == NT_A - 1 and phases in ("G", "full"):
                        ag(t1loc["A"], tab1["A"])   # overlap with B-chunk finalize

            # ---- AllGather table1 (B chunk) ----
            if phases in ("G", "full"):
                ag(t1loc["B"], tab1["B"])

            if phases == "full":
                edge_phase(1)

            # ---- phase E: finalize layer-1 ----
            for u in (range(NCHUNK) if phases == "full" else []):
                gL = gpool.tile([128, 8, RU1], dt.bfloat16, tag="hg")
                gH = gpool.tile([128, 8, RU1], dt.bfloat16, tag="adg")
                for h, g in (("A", gL), ("B", gH)):
                    rix = pool.tile([128, 64], dt.int16, tag="rix")
                    nc.sync.dma_start(out=rix[:],
                                      in_=streams[h]["rowchunks"][u * 128:(u + 1) * 128, :])
                    gather(g[:], seq1[h][:], rix[:], 1024, RU1)
                for tt in range(8):
                    t = u * 8 + tt
                    if t >= NT_C:
                        break
                    r0 = t * 128
                    cnt = 128 if t < NT_C - 1 else LAST_C
                    o = pool.tile([128, OW1], dt.bfloat16, tag="o1")
                    nc.vector.tensor_tensor(out=o[:], in0=gL[:, tt, 0:OW1],
                                            in1=gH[:, tt, 0:OW1], op=OP.add)
                    rec = pool.tile([128, 1], dt.float32, tag="rec1")
                    nc.vector.reciprocal(out=rec[:], in_=o[:, 2:3])
                    rec2 = pool.tile([128, 2], dt.bfloat16, tag="rec12")
                    nc.vector.tensor_copy(out=rec2[:], in_=rec[:].to_broadcast([128, 2]))
                    res = pool.tile([128, HID], dt.bfloat16, tag="res")
                    nc.vector.tensor_tensor(
                        out=res[:].rearrange("p (c b) -> p c b", b=2),
                        in0=o[:, 4:4 + HID].rearrange("p (c b) -> p c b", b=2),
                        in1=rec2[:].rearrange("p b -> p b ()").rearrange("p b u -> p u b")
                            .to_broadcast([128, HID // 2, 2]),
                        op=OP.mult)
                    resf = pool.tile([128, HID], dt.float32, tag="resf")
                    nc.vector.tensor_tensor(out=resf[:], in0=res[:], in1=bias1tt[:], op=OP.add)
                    nc.scalar.dma_start(out=out_f[r0:r0 + cnt, :], in_=resf[0:cnt, :])

            if phases != "full":
                for t in range(NT_C):
                    r0 = t * 128
                    cnt = 128 if t < NT_C - 1 else LAST_C
                    zf32 = cpool.tile([128, HID], dt.float32)
                    nc.vector.memset(zf32[:], 0.0)
                    nc.sync.dma_start(out=out_f[r0:r0 + cnt, :], in_=zf32[0:cnt, :])

    nc.compile()
    return nc


def kernel(**inputs):
    import os
    from concourse import bass_utils
    in_maps, n_sb = host_prepare(inputs)
    phases = os.environ.get("KGAT_PHASES", "full")
    key = (n_sb["A"], n_sb["B"], phases)
    if key not in _prog_cache:
        _prog_cache[key] = build_program(n_sb, phases)
    nc = _prog_cache[key]
    res = bass_utils.run_bass_kernel_spmd(nc, in_maps, core_ids=list(range(N_CORES)))
    out = np.concatenate([np.asarray(res.results[c]["out"]) for c in range(N_CORES)], axis=0)
    return out.astype(np.float32)


# revision 38
# speedup vs baseline: 1.0973x; 1.0371x over previous
"""Trainium2 Bass kernel for 2-layer GAT (nn_GAT_47957604827269).

Strategy: partition nodes across 8 cores by dst range. Per layer:
  - per-core table slice build (local x^T tiles -> PE matmuls), AllGather to
    a full per-node feature table in DRAM: row = [as | (1,1,h)*H] in bf16,
    256B-aligned rows for the dma_gather ucode op.
  - per-edge gather (dma_gather, int16 idx -> edges split by table row <
    32768), attention weights w = exp(leaky_relu(as[src] + ad[dst])) on-chip
    (ad gathered from a core-local bf16 table), aggregation via one-hot
    slot-matmul on the PE: B.T @ (w * [1|1|h]) giving per-node numerators and
    (via the duplicated ones columns) denominators in one pass.
  - block results land contiguously in DRAM ("seq" buffers, bf16); the
    finalize pass gathers each node's L/H partial rows, adds, normalizes.

DVE ops use 16-bit dtypes with pair-replicated scalars so the per-element
broadcast multiplies hit the DVE 2x packed mode.
"""
import numpy as np
import ml_dtypes

BF16 = ml_dtypes.bfloat16

# ---- problem constants (hardcoded per contract) ----
N = 50000
F_IN = 128
HID = 64
H0 = 4
N_CORES = 8
NPC = N // N_CORES            # 6250
NLOCPAD = 6272                # 49*128: padded rows per core (table row space)
CHA = 3200                    # chunk-A rows per core (25 tiles)
CHB = 3072                    # chunk-B rows per core (24 tiles)
NRA = CHA * N_CORES           # 25600 rows in table chunk A (< 2^15 for int16)
NRB = CHB * N_CORES           # 24576 rows in table chunk B
SLOTS = 64
TRASH = SLOTS - 1             # 63
GEO = {"A": dict(BLK=512, SUB=4, SBB=8),    # 4096 edges / superblock
       "B": dict(BLK=512, SUB=4, SBB=8)}
SBE = 4096
SUBMAX = 6
RU0 = 384                     # bf16 units per table0 row (768B); 268 used
RU1 = 128                     # table1 row units (256B); 68 used
ADB = 128                     # ad table row bf16 units (256B)
OW0 = 264                     # seq0 used cols (bf16), row stride RU0
OW1 = 68                      # seq1 used cols (bf16), row stride RU1
HW0 = 66                      # layer-0 per-head block: [1,1,h*64]
NT_C = NLOCPAD // 128         # 49
LAST_C = NPC - 48 * 128       # 106
NCHUNK = 7                    # finalize gather chunks of 1024 nodes

_prog_cache = {}


def _wrap16(idx, pad_to=None):
    """ucode idx layout: idx i at [i%16, i//16], replicated to 128 partitions."""
    idx = np.asarray(idx, np.int16)
    if pad_to is not None and len(idx) < pad_to:
        idx = np.concatenate([idx, np.zeros(pad_to - len(idx), np.int16)])
    n = len(idx)
    a = idx.reshape(n // 16, 16).T.copy()
    return np.tile(a, (8, 1))


def _pack_half(ss, dd, geo):
    """Greedy-pack edges (dst-sorted local) into BLK-edge / 63-slot blocks."""
    BLK = geo["BLK"]
    blocks = []
    if len(ss):
        uniq, starts = np.unique(dd, return_index=True)
        ends = np.append(starts[1:], len(dd))
        cur_s, cur_nodes, cur_slot = [], [], []
        for nd, st, en in zip(uniq, starts, ends):
            deg = en - st
            if len(cur_s) + deg > BLK or len(cur_nodes) >= TRASH:
                blocks.append((cur_s, cur_slot, cur_nodes))
                cur_s, cur_nodes, cur_slot = [], [], []
            sl = len(cur_nodes)
            cur_nodes.append(nd)
            cur_s.extend(ss[st:en])
            cur_slot.extend([sl] * deg)
        if cur_s:
            blocks.append((cur_s, cur_slot, cur_nodes))
    return blocks


def _pack_core(src, dst, core):
    lo, hi = core * NPC, (core + 1) * NPC
    m = (dst >= lo) & (dst < hi)
    s = src[m]
    d_loc = dst[m] - lo
    order = np.argsort(d_loc, kind="stable")
    s, d_loc = s[order], d_loc[order]
    sc, so = s // NPC, s % NPC          # owning core, local offset
    in_a = so < CHA
    out = {}
    for half, sel in (("A", in_a), ("B", ~in_a)):
        ss = np.where(in_a, sc * CHA + so, sc * CHB + (so - CHA))[sel]
        out[half] = _pack_half(ss, d_loc[sel], GEO[half])
    return out


def _streams_for_half(blocks, n_sb_target, geo):
    BLK, SBB = geo["BLK"], geo["SBB"]
    nbt = n_sb_target * SBB
    src_b = np.zeros((nbt, BLK), np.int32)
    slot_b = np.full((nbt, BLK), TRASH, np.int32)
    dloc_b = np.zeros((nbt, BLK), np.int32)
    rowpos = np.full(NLOCPAD + 1024, nbt * SLOTS, np.int32)  # default: zero row
    for j, (s_, sl_, nds) in enumerate(blocks):
        k = len(s_)
        src_b[j, :k] = s_
        slot_b[j, :k] = sl_
        nda = np.asarray(nds, np.int32)
        dloc_b[j, :k] = nda[np.asarray(sl_, np.int32)]
        rowpos[nda] = j * SLOTS + np.arange(len(nds), dtype=np.int32)
    src_sb = src_b.reshape(n_sb_target, SBB * BLK)
    dloc_sb = dloc_b.reshape(n_sb_target, SBB * BLK)
    hidx = np.stack([_wrap16(r.astype(np.int16)) for r in src_sb])
    adidx = np.stack([_wrap16(r.astype(np.int16)) for r in dloc_sb])
    # paired bf16 slot stream: [n_sb, 128, SBB*SUB, 2]
    slotb = (slot_b.reshape(n_sb_target, SBB * (BLK // 128), 128)
             .transpose(0, 2, 1))                 # [n_sb, 128, SBB*SUB]
    slotb2 = np.repeat(slotb.astype(BF16), 2, axis=2).reshape(n_sb_target, 128, -1)
    # finalize gather idx: chunks of 1024 node ids
    rows = rowpos[:NCHUNK * 1024].astype(np.int16)
    rowchunks = np.stack([_wrap16(rows[u * 1024:(u + 1) * 1024])
                          for u in range(NCHUNK)])
    return dict(hidx=hidx.astype(np.int16), adidx=adidx.astype(np.int16),
                slotb2=slotb2, rowchunks=rowchunks.astype(np.int16))


def host_prepare(inputs):
    x = np.ascontiguousarray(np.asarray(inputs["x"], np.float32))
    ei = np.asarray(inputs["edge_index"], np.int32)
    W0 = np.asarray(inputs["W0"], np.float32)
    as0 = np.asarray(inputs["att_src0"], np.float32)
    ad0 = np.asarray(inputs["att_dst0"], np.float32)
    b0 = np.asarray(inputs["bias0"], np.float32)
    W1 = np.asarray(inputs["W1"], np.float32)
    as1 = np.asarray(inputs["att_src1"], np.float32)
    ad1 = np.asarray(inputs["att_dst1"], np.float32)
    b1 = np.asarray(inputs["bias1"], np.float32)

    A_s0 = np.einsum("ihc,hc->ih", W0.reshape(F_IN, H0, HID), as0).astype(np.float32)
    A_d0 = np.einsum("ihc,hc->ih", W0.reshape(F_IN, H0, HID), ad0).astype(np.float32)
    A_sd0 = np.concatenate([A_s0, A_d0], axis=1)  # [F_IN, 8]
    A_sd1 = np.stack([
        np.einsum("ihc,hc->ih", W1.reshape(H0 * HID, 1, HID), as1)[:, 0],
        np.einsum("ihc,hc->ih", W1.reshape(H0 * HID, 1, HID), ad1)[:, 0],
    ], axis=1).astype(np.float32)

    loop = np.arange(N, dtype=np.int32)
    src = np.concatenate([ei[0], loop])
    dst = np.concatenate([ei[1], loop])

    packs = [_pack_core(src, dst, c) for c in range(N_CORES)]
    n_sb = {h: max((len(p[h]) + GEO[h]["SBB"] - 1) // GEO[h]["SBB"] for p in packs)
            for h in ("A", "B")}

    common = {
        "W0b": W0.astype(BF16),
        "A_sd0": A_sd0.astype(BF16),
        "W1b": W1.astype(BF16),
        "A_sd1b": A_sd1.astype(BF16),
        "bias0t": np.tile(b0[None, :], (128, 1)).astype(BF16),
        "bias1t": np.tile(b1[None, :], (128, 1)).astype(BF16),
        "iota64": np.tile(np.arange(SLOTS, dtype=np.float32), (128, SUBMAX, 1))
                    .reshape(128, SUBMAX * SLOTS).astype(BF16),
        "ones_bf": np.ones((128, 1), BF16),
        "identb": np.eye(128, dtype=np.float32).astype(BF16),
    }
    in_maps = []
    for c in range(N_CORES):
        d = dict(common)
        xl = np.zeros((NLOCPAD, F_IN), np.float32)
        xl[:NPC] = x[c * NPC:(c + 1) * NPC]
        d["xT"] = np.ascontiguousarray(xl.T).astype(BF16)  # [F_IN, NLOCPAD]
        for half in ("A", "B"):
            st = _streams_for_half(packs[c][half], n_sb[half], GEO[half])
            for k, v in st.items():
                d[f"{k}_{half}"] = v.reshape(-1, v.shape[-1])
        in_maps.append(d)
    return in_maps, n_sb


# ----------------------------------------------------------------------------
# bass program
# ----------------------------------------------------------------------------

def build_program(n_sb, phases="full"):
    import concourse.bass as bass
    import concourse.bacc as bacc
    import concourse.tile as tile
    import concourse.mybir as mybir
    dt = mybir.dt

    import os as _os
    GCH = int(_os.environ.get("KGAT_GCH", "1024"))
    nc = bacc.Bacc("TRN2", target_bir_lowering=False, debug=False,
                   enable_asserts=False, num_devices=N_CORES,
                   num_swdge_queues=4,
                   dynamic_dma_scratch_size=16384 * (GCH // 1024))

    _gq = [0]

    def gather(out_ap, in_ap, idxs_ap, num_idxs, elem_size):
        # dma_gather corrupts above the SWDGE ring capacity; chunk at GCH.
        done = 0
        while done < num_idxs:
            ch = min(GCH, num_idxs - done)
            assert ch % 128 == 0
            nc.gpsimd.dma_gather(
                out_ap=out_ap[:, done // 128:(done + ch) // 128, :],
                in_ap=in_ap,
                idxs_ap=idxs_ap[:, done // 16:(done + ch) // 16],
                num_idxs=ch, num_idxs_reg=ch, elem_size=elem_size,
                queue_num=_gq[0] % 4)
            _gq[0] += 1
            done += ch

    def inp(name, shape, dtype):
        return nc.dram_tensor(name, shape, dtype, kind="ExternalInput").ap()

    xT = inp("xT", [F_IN, NLOCPAD], dt.bfloat16)
    W0b = inp("W0b", [F_IN, H0 * HID], dt.bfloat16)
    A_sd0 = inp("A_sd0", [F_IN, 2 * H0], dt.bfloat16)
    W1b = inp("W1b", [H0 * HID, HID], dt.bfloat16)
    A_sd1b = inp("A_sd1b", [H0 * HID, 2], dt.bfloat16)
    bias0t = inp("bias0t", [128, H0 * HID], dt.bfloat16)
    bias1t = inp("bias1t", [128, HID], dt.bfloat16)
    iota64 = inp("iota64", [128, SUBMAX * SLOTS], dt.bfloat16)
    ones_bf = inp("ones_bf", [128, 1], dt.bfloat16)
    identb = inp("identb", [128, 128], dt.bfloat16)
    streams = {}
    nrows = {}
    for half in ("A", "B"):
        ns, SBB, SUBh = n_sb[half], GEO[half]["SBB"], GEO[half]["SUB"]
        streams[half] = dict(
            hidx=inp(f"hidx_{half}", [ns * 128, SBE // 16], dt.int16),
            adidx=inp(f"adidx_{half}", [ns * 128, SBE // 16], dt.int16),
            slotb2=inp(f"slotb2_{half}", [ns * 128, SBB * SUBh * 2], dt.bfloat16),
            rowchunks=inp(f"rowchunks_{half}", [NCHUNK * 128, 64], dt.int16),
        )
        nrows[half] = ns * SBB * SLOTS + 128   # + zero block
    out_f = nc.dram_tensor("out", [NPC, HID], dt.float32, kind="ExternalOutput").ap()

    CH = {"A": CHA, "B": CHB}
    NR = {"A": NRA, "B": NRB}
    t0loc = {h: nc.dram_tensor(f"t0loc{h}", [CH[h], RU0], dt.bfloat16).ap()
             for h in ("A", "B")}
    tab0 = {h: nc.dram_tensor(f"tab0{h}", [NR[h], RU0], dt.bfloat16,
                              addr_space="Shared").ap() for h in ("A", "B")}
    ad0_loc = nc.dram_tensor("ad0_loc", [NLOCPAD, ADB], dt.bfloat16).ap()
    seq0 = {h: nc.dram_tensor(f"seq0{h}", [nrows[h], RU0], dt.bfloat16).ap()
            for h in ("A", "B")}
    t1loc = {h: nc.dram_tensor(f"t1loc{h}", [CH[h], RU1], dt.bfloat16).ap()
             for h in ("A", "B")}
    tab1 = {h: nc.dram_tensor(f"tab1{h}", [NR[h], RU1], dt.bfloat16,
                              addr_space="Shared").ap() for h in ("A", "B")}
    ad1_loc = nc.dram_tensor("ad1_loc", [NLOCPAD, ADB], dt.bfloat16).ap()
    seq1 = {h: nc.dram_tensor(f"seq1{h}", [nrows[h], RU1], dt.bfloat16).ap()
            for h in ("A", "B")}
    NT_A = CHA // 128   # 25 tiles in chunk A

    AF = mybir.ActivationFunctionType
    OP = mybir.AluOpType

    def pair_bcast(ap_pk2, outer, inner):
        """[128, outer, 2] -> broadcast AP [128, outer, inner, 2]."""
        return (ap_pk2.rearrange("p a b -> p a b ()")
                .rearrange("p a b u -> p a u b")
                .to_broadcast([128, outer, inner, 2]))

    with tile.TileContext(nc) as tc:
        with (
            tc.tile_pool(name="const", bufs=1) as cpool,
            tc.tile_pool(name="sbuf", bufs=3) as pool,
            tc.tile_pool(name="gath", bufs=3) as gpool,
            tc.tile_pool(name="psum", bufs=2, space="PSUM") as psum,
            tc.tile_pool(name="psB", bufs=2, space="PSUM") as psumB,
        ):
            identt = cpool.tile([128, 128], dt.bfloat16)
            nc.sync.dma_start(out=identt[:], in_=identb[:])
            W0bt = cpool.tile([128, H0 * HID], dt.bfloat16)
            nc.sync.dma_start(out=W0bt[:], in_=W0b[:])
            A_sd0t = cpool.tile([128, 2 * H0], dt.bfloat16)
            nc.sync.dma_start(out=A_sd0t[:], in_=A_sd0[:])
            W1bt = cpool.tile([128, 2, HID], dt.bfloat16)
            nc.sync.dma_start(out=W1bt[:], in_=W1b[:].rearrange("(a p) d -> p a d", p=128))
            A_sd1t = cpool.tile([128, 2, 2], dt.bfloat16)
            nc.sync.dma_start(out=A_sd1t[:], in_=A_sd1b[:].rearrange("(a p) d -> p a d", p=128))
            bias0tt = cpool.tile([128, H0 * HID], dt.bfloat16)
            nc.sync.dma_start(out=bias0tt[:], in_=bias0t[:])
            bias1tt = cpool.tile([128, HID], dt.bfloat16)
            nc.sync.dma_start(out=bias1tt[:], in_=bias1t[:])
            iota64t = cpool.tile([128, SUBMAX, SLOTS], dt.bfloat16)
            nc.sync.dma_start(out=iota64t[:],
                              in_=iota64[:].rearrange("p (a b) -> p a b", a=SUBMAX))
            ones_bft = cpool.tile([128, 1], dt.bfloat16)
            nc.sync.dma_start(out=ones_bft[:], in_=ones_bf[:])
            zeroB = cpool.tile([128, RU0], dt.bfloat16)
            nc.vector.memset(zeroB[:], 0.0)

            # zero rows at tail of each seq buffer
            for h in ("A", "B"):
                nc.sync.dma_start(out=seq0[h][nrows[h] - 128:, :], in_=zeroB[:])
                nc.sync.dma_start(out=seq1[h][nrows[h] - 128:, :], in_=zeroB[:, :RU1])

            # ---- phase A: local table0 slice + ad0 table (sharded) ----
            def phase_a_tile(t):
                r0 = t * 128
                xt = pool.tile([128, 128], dt.bfloat16, tag="xt")
                nc.sync.dma_start(out=xt[:], in_=xT[:, r0:r0 + 128])
                psH = psum.tile([128, H0 * HID], dt.float32, tag="psH")
                nc.tensor.matmul(out=psH[:], lhsT=xt[:], rhs=W0bt[:], start=True, stop=True)
                psA = psum.tile([128, 2 * H0], dt.float32, tag="psA")
                nc.tensor.matmul(out=psA[:], lhsT=xt[:], rhs=A_sd0t[:], start=True, stop=True)
                stag = pool.tile([128, RU0], dt.bfloat16, tag="stag")
                nc.vector.tensor_copy(out=stag[:, 0:H0], in_=psA[:, 0:H0])
                sv = stag[:, H0:H0 + H0 * HW0].rearrange("p (h u) -> p h u", h=H0)
                nc.vector.tensor_copy(
                    out=sv[:, :, 0:2],
                    in_=ones_bft[:].rearrange("p u -> p u ()").to_broadcast([128, H0, 2]))
                nc.vector.tensor_copy(
                    out=sv[:, :, 2:HW0],
                    in_=psH[:].rearrange("p (h u) -> p h u", h=H0))
                if t < NT_A:
                    nc.scalar.dma_start(out=t0loc["A"][r0:r0 + 128, :], in_=stag[:])
                else:
                    nc.scalar.dma_start(out=t0loc["B"][r0 - CHA:r0 - CHA + 128, :],
                                      in_=stag[:])
                adst = pool.tile([128, ADB], dt.bfloat16, tag="adst")
                nc.vector.tensor_copy(out=adst[:, 0:H0], in_=psA[:, H0:2 * H0])
                nc.scalar.dma_start(out=ad0_loc[r0:r0 + 128, :], in_=adst[:])

            def ag(src_ap, dst_ap):
                nc.gpsimd.collective_compute(
                    "AllGather", OP.bypass, replica_groups=[list(range(N_CORES))],
                    ins=[src_ap.opt()], outs=[dst_ap.opt()])

            if phases != "empty":
                for t in range(NT_A):
                    phase_a_tile(t)
                if phases in ("B", "C", "G", "full"):
                    ag(t0loc["A"], tab0["A"])
                for t in range(NT_A, NT_C):
                    phase_a_tile(t)
                if phases in ("B", "C", "G", "full"):
                    ag(t0loc["B"], tab0["B"])

            # ---- edge phase ----
            def edge_phase(layer):
                if layer == 0:
                    tabs, ad_loc, seqT, ru, nheads = tab0, ad0_loc, seq0, RU0, H0
                    as_u, hw, mo, ow = H0, HW0, H0, OW0
                else:
                    tabs, ad_loc, seqT, ru, nheads = tab1, ad1_loc, seq1, RU1, 1
                    as_u, hw, mo, ow = 1, OW1, 0, OW1
                import os as _os
                _halves = _os.environ.get("KGAT_HALVES", "AB")
                _maxsb = int(_os.environ.get("KGAT_MAXSB", "9999"))
                for half in [h for h in ("A", "B") if h in _halves]:
                    SUBh, SBB = GEO[half]["SUB"], GEO[half]["SBB"]
                    tab = tabs[half]
                    st = streams[half]
                    for s in range(min(n_sb[half], _maxsb)):
                        hix = pool.tile([128, SBE // 16], dt.int16, tag="hix")
                        nc.sync.dma_start(out=hix[:], in_=st["hidx"][s * 128:(s + 1) * 128, :])
                        aix = pool.tile([128, SBE // 16], dt.int16, tag="aix")
                        nc.sync.dma_start(out=aix[:], in_=st["adidx"][s * 128:(s + 1) * 128, :])
                        slt = pool.tile([128, SBB * SUBh, 2], dt.bfloat16, tag="slt")
                        nc.sync.dma_start(
                            out=slt[:],
                            in_=st["slotb2"][s * 128:(s + 1) * 128, :]
                                .rearrange("p (a b) -> p a b", b=2))

                        hg = gpool.tile([128, SBE // 128, ru], dt.bfloat16, tag="hg")
                        gather(hg[:], tab, hix[:], SBE, ru)
                        adg = gpool.tile([128, SBE // 128, ADB], dt.bfloat16, tag="adg")
                        gather(adg[:], ad_loc[:], aix[:], SBE, ADB)

                        # superblock-wide attention weights + one-hot build:
                        # one instruction each instead of one per block.
                        NS = SBB * SUBh
                        ev = pool.tile([128, NS, nheads], dt.bfloat16, tag="ev")
                        nc.vector.tensor_tensor(
                            out=ev[:], in0=hg[:, :, 0:nheads],
                            in1=adg[:, :, 0:nheads], op=OP.add)
                        tv = pool.tile([128, NS, nheads], dt.bfloat16, tag="tv")
                        nc.vector.tensor_scalar_mul(out=tv[:], in0=ev[:], scalar1=0.2)
                        nc.vector.tensor_tensor(out=tv[:], in0=tv[:], in1=ev[:], op=OP.max)
                        wv = pool.tile([128, NS, nheads], dt.bfloat16, tag="wv")
                        nc.scalar.activation(out=wv[:], in_=tv[:], func=AF.Exp)
                        wv2 = pool.tile([128, NS, nheads, 2], dt.bfloat16, tag="wv2")
                        nc.vector.tensor_copy(
                            out=wv2[:],
                            in_=wv[:].rearrange("p a h -> p a h ()")
                                .to_broadcast([128, NS, nheads, 2]))
                        B8 = pool.tile([128, NS, SLOTS], dt.bfloat16, tag="B8")
                        nc.vector.tensor_tensor(
                            out=B8[:].rearrange("p a (c b) -> p a c b", b=2),
                            in0=pair_bcast(slt[:], NS, SLOTS // 2),
                            in1=iota64t[:, 0, :].rearrange("p (c b) -> p c b", b=2)
                                .rearrange("p c b -> p () c b")
                                .to_broadcast([128, NS, SLOTS // 2, 2]),
                            op=OP.is_equal)

                        # superblock-wide weighted rhs: one multiply per head
                        rhs = pool.tile([128, NS, nheads * hw], dt.bfloat16, tag="rhs")
                        for hh in range(nheads):
                            o = mo + hw * hh
                            nc.vector.tensor_tensor(
                                out=rhs[:, :, hw * hh:hw * hh + hw]
                                    .rearrange("p a (c b) -> p a c b", b=2),
                                in0=hg[:, :, o:o + hw]
                                    .rearrange("p a (c b) -> p a c b", b=2),
                                in1=pair_bcast(wv2[:, :, hh, :], NS, hw // 2),
                                op=OP.mult)

                        stage = pool.tile([128, SBB // 2, ru], dt.bfloat16, tag="stage")
                        for b in range(SBB):
                            g0 = b * SUBh
                            m = 64 * (b % 2)
                            if b % 2 == 0:
                                ps = psumB.tile([128, ow], dt.float32, tag="psB")
                            for k in range(SUBh):
                                nc.tensor.matmul(
                                    out=ps[m:m + 64, 0:nheads * hw],
                                    lhsT=B8[:, g0 + k, :], rhs=rhs[:, g0 + k, :],
                                    start=(k == 0), stop=(k == SUBh - 1))
                            if b % 2 == 1:
                                c = b // 2
                                nc.vector.tensor_copy(out=stage[:, c, 0:ow], in_=ps[:, 0:ow])
                        r0 = s * SBB * SLOTS
                        nc.scalar.dma_start(
                            out=seqT[half][r0:r0 + SBB * SLOTS, :]
                                .rearrange("(c p) u -> p c u", p=128),
                            in_=stage[:])

            if phases in ("B", "C", "G", "full"):
                edge_phase(0)

            # ---- phase C: finalize layer-0, build table1 local slice ----
            for u in (range(NCHUNK) if phases in ("C", "G", "full") else []):
                gL = gpool.tile([128, 8, RU0], dt.bfloat16, tag="hg")
                gH = gpool.tile([128, 8, RU0], dt.bfloat16, tag="adg")
                for h, g in (("A", gL), ("B", gH)):
                    rix = pool.tile([128, 64], dt.int16, tag="rix")
                    nc.sync.dma_start(out=rix[:],
                                      in_=streams[h]["rowchunks"][u * 128:(u + 1) * 128, :])
                    gather(g[:], seq0[h][:], rix[:], 1024, RU0)
                for tt in range(8):
                    t = u * 8 + tt
                    if t >= NT_C:
                        break
                    r0 = t * 128
                    cnt = 128 if t < NT_C - 1 else LAST_C
                    o = pool.tile([128, OW0], dt.bfloat16, tag="oC")
                    nc.vector.tensor_tensor(out=o[:], in0=gL[:, tt, 0:OW0],
                                            in1=gH[:, tt, 0:OW0], op=OP.add)
                    ov = o[:].rearrange("p (h u) -> p h u", h=H0)
                    rec = pool.tile([128, H0], dt.float32, tag="rec")
                    nc.vector.reciprocal(out=rec[:],
                                         in_=ov[:, :, 0:1].rearrange("p h u -> p (h u)"))
                    rec2 = pool.tile([128, H0, 2], dt.bfloat16, tag="rec2")
                    nc.vector.tensor_copy(
                        out=rec2[:],
                        in_=rec[:].rearrange("p h -> p h ()").to_broadcast([128, H0, 2]))
                    z = pool.tile([128, H0, HID], dt.bfloat16, tag="z")
                    nc.vector.tensor_tensor(
                        out=z[:].rearrange("p h (c b) -> p h c b", b=2),
                        in0=ov[:, :, 2:HW0].rearrange("p h (c b) -> p h c b", b=2),
                        in1=pair_bcast(rec2[:], H0, HID // 2),
                        op=OP.mult)
                    zf = z[:].rearrange("p h u -> p (h u)")
                    nc.vector.tensor_tensor(out=zf, in0=zf, in1=bias0tt[:], op=OP.add)
                    zm = pool.tile([128, H0 * HID], dt.bfloat16, tag="zm")
                    nc.vector.tensor_scalar_min(out=zm[:], in0=zf, scalar1=0.0)
                    qe = pool.tile([128, H0 * HID], dt.bfloat16, tag="qe")
                    nc.scalar.activation(out=qe[:], in_=zm[:], func=AF.Exp)
                    nc.vector.tensor_scalar(out=zf, in0=zf, scalar1=0.0, scalar2=-1.0,
                                            op0=OP.max, op1=OP.add)
                    nc.vector.tensor_tensor(out=zf, in0=zf, in1=qe[:], op=OP.add)
                    psH1 = psum.tile([128, HID], dt.float32, tag="psH")
                    psA1 = psum.tile([128, 2], dt.float32, tag="psA")
                    for ch in range(2):
                        psT = psum.tile([128, 128], dt.bfloat16, tag="psT")
                        nc.tensor.transpose(out=psT[:],
                                            in_=zf[:, 128 * ch:128 * ch + 128],
                                            identity=identt[:])
                        zTb = pool.tile([128, 128], dt.bfloat16, tag="zTb")
                        nc.vector.tensor_copy(out=zTb[:], in_=psT[:])
                        nc.tensor.matmul(out=psH1[:], lhsT=zTb[:], rhs=W1bt[:, ch, :],
                                         start=(ch == 0), stop=(ch == 1))
                        nc.tensor.matmul(out=psA1[:], lhsT=zTb[:], rhs=A_sd1t[:, ch, :],
                                         start=(ch == 0), stop=(ch == 1))
                    t1s = pool.tile([128, RU1], dt.bfloat16, tag="t1s")
                    nc.vector.tensor_copy(
                        out=t1s[:, 0:2],
                        in_=psA1[:, 0:1].to_broadcast([128, 2]))
                    nc.vector.tensor_copy(
                        out=t1s[:, 2:4],
                        in_=ones_bft[:].to_broadcast([128, 2]))
                    nc.vector.tensor_copy(out=t1s[:, 4:4 + HID], in_=psH1[:])
                    if t < NT_A:
                        nc.scalar.dma_start(out=t1loc["A"][r0:r0 + cnt, :],
                                          in_=t1s[0:cnt, :])
                    else:
                        nc.scalar.dma_start(out=t1loc["B"][r0 - CHA:r0 - CHA + cnt, :],
                                          in_=t1s[0:cnt, :])
                    a1s = pool.tile([128, ADB], dt.bfloat16, tag="adst")
                    nc.vector.tensor_copy(out=a1s[:, 0:1], in_=psA1[:, 1:2])
                    nc.scalar.dma_start(out=ad1_loc[r0:r0 + 128, :], in_=a1s[:])
                    if t == NT_A - 1 and phases in ("G", "full"):
                        ag(t1loc["A"], tab1["A"])   # overlap with B-chunk finalize

            # ---- AllGather table1 (B chunk) ----
            if phases in ("G", "full"):
                ag(t1loc["B"], tab1["B"])

            if phases == "full":
                edge_phase(1)

            # ---- phase E: finalize layer-1 ----
            for u in (range(NCHUNK) if phases == "full" else []):
                gL = gpool.tile([128, 8, RU1], dt.bfloat16, tag="hg")
                gH = gpool.tile([128, 8, RU1], dt.bfloat16, tag="adg")
                for h, g in (("A", gL), ("B", gH)):
                    rix = pool.tile([128, 64], dt.int16, tag="rix")
                    nc.sync.dma_start(out=rix[:],
                                      in_=streams[h]["rowchunks"][u * 128:(u + 1) * 128, :])
                    gather(g[:], seq1[h][:], rix[:], 1024, RU1)
                for tt in range(8):
                    t = u * 8 + tt
                    if t >= NT_C:
                        break
                    r0 = t * 128
                    cnt = 128 if t < NT_C - 1 else LAST_C
                    o = pool.tile([128, OW1], dt.bfloat16, tag="o1")
                    nc.vector.tensor_tensor(out=o[:], in0=gL[:, tt, 0:OW1],
                                            in1=gH[:, tt, 0:OW1], op=OP.add)
                    rec = pool.tile([128, 1], dt.float32, tag="rec1")
                    nc.vector.reciprocal(out=rec[:], in_=o[:, 2:3])
                    rec2 = pool.tile([128, 2], dt.bfloat16, tag="rec12")
                    nc.vector.tensor_copy(out=rec2[:], in_=rec[:].to_broadcast([128, 2]))
                    res = pool.tile([128, HID], dt.bfloat16, tag="res")
                    nc.vector.tensor_tensor(
                        out=res[:].rearrange("p (c b) -> p c b", b=2),
                        in0=o[:, 4:4 + HID].rearrange("p (c b) -> p c b", b=2),
                        in1=rec2[:].rearrange("p b -> p b ()").rearrange("p b u -> p u b")
                            .to_broadcast([128, HID // 2, 2]),
                        op=OP.mult)
                    resf = pool.tile([128, HID], dt.float32, tag="resf")
                    nc.vector.tensor_tensor(out=resf[:], in0=res[:], in1=bias1tt[:], op=OP.add)
                    nc.scalar.dma_start(out=out_f[r0:r0 + cnt, :], in_=resf[0:cnt, :])

            if phases != "full":
                for t in range(NT_C):
                    r0 = t * 128
                    cnt = 128 if t < NT_C - 1 else LAST_C
                    zf32 = cpool.tile([128, HID], dt.float32)
                    nc.vector.memset(zf32[:], 0.0)
                    nc.sync.dma_start(out=out_f[r0:r0 + cnt, :], in_=zf32[0:cnt, :])

    nc.compile()
    return nc


def kernel(**inputs):
    import os
    from concourse import bass_utils
    in_maps, n_sb = host_prepare(inputs)
    phases = os.environ.get("KGAT_PHASES", "full")
    key = (n_sb["A"], n_sb["B"], phases)
    if key not in _prog_cache:
        _prog_cache[key] = build_program(n_sb, phases)
    nc = _prog_cache[key]
    res = bass_utils.run_bass_kernel_spmd(nc, in_maps, core_ids=list(range(N_CORES)))
    out = np.concatenate([np.asarray(res.results[c]["out"]) for c in range(N_CORES)], axis=0)
    return out.astype(np.float32)
